# revision 35
# baseline (speedup 1.0000x reference)
"""Distributed multi-head attention kernel for 8 TRN2 NeuronCores.

Problem: B=2, N=2048, C=768, H=12 heads of dim 64.
  q = x @ Wq.T ; k = x @ Wk.T ; v = x @ Wv.T      (per-head split)
  out = softmax(q k^T / 8) v                        (full N^2 attention)
  y = concat_heads(out) @ Wo.T + bo

Sharding: 24 (batch, head) pairs -> 3 per core.  Core i owns batch i//4 and
heads 3*(i%4)..3*(i%4)+2.  Projections + attention are fully local (weights
row-sliced on the host).  A per-head 8-way AllToAll redistributes the
context so core i owns query rows 256*i..256*(i+1) of BOTH batches, and the
output projection accumulates PER A2A ROUND (4 global heads at a time, Wo
row-permuted dh-major on the host), leaving only the last head's A2A on the
critical path -- and that one is split into two d-half pieces pipelined
against its own output projection.

Schedule: the attention inner loop is ACT(exp)-bound (~1.15us per key-block
covering 2 q-chunks, [128,1024] exp tiles on a 2-deep PSUM ring).  All
other PE work (later heads' Q/K projections, per-round output projection,
reciprocal broadcasts) is interleaved one small "extra" per iteration so
the PE stream stays dense without out-running the ACT.  Softmax
normalization computes 1/d = exp(-ln(d)) on the ACT (ln+exp+identity share
one activation table set, pinned at compile time so walrus emits a single
ACT_TABLE_LOAD) and broadcasts the reciprocal row across partitions with a
tiny [2,64]-selector matmul -- no DRAM round-trip, no slow DVE reciprocal.
A dummy 1KB AllToAll during the lead-in absorbs the fabric warm-up that
otherwise makes the first real collective run at a third of link rate.
"""

import numpy as np
import ml_dtypes

import concourse.mybir as mybir
import concourse.tile as tile
from concourse import bacc
from concourse.bass_utils import run_bass_kernel_spmd

B, N, C, H, HD = 2, 2048, 768, 12, 64
SCALE = HD ** -0.5          # 0.125
P = 128
CB = C // P                 # 6 contraction blocks of 128 over channels
KB = N // P                 # 16 key blocks
QCH = 512                   # query chunk (max moving free dim)
HPC = 3                     # heads per core
NCORES = 8
VW = HPC * (HD + 1)         # 195: v columns per key-block (3 heads + ones col)
RQ = N // NCORES            # 256 query rows per core per batch after A2A
PVLAG = 6                   # PV trails scores by this many key-blocks
# iterations from A2A staging to its out-proj work (waits for the
# collective to land so the PE stream never head-of-line blocks on it)
OUTPROJ_DELAY = {0: 34, 1: 26}

f32 = mybir.dt.float32
bf16 = mybir.dt.bfloat16
Exp = mybir.ActivationFunctionType.Exp
Log = mybir.ActivationFunctionType.Ln
Identity = mybir.ActivationFunctionType.Identity

# head -> (block, partition offset) inside qT_sb / kT_sb [128, 2*2048].
HOFF = {0: (0, 0), 1: (0, 64), 2: (1, 0)}
# wqkT host column order: [q0 q1 | k0 k1 | q2 | k2]
# (col offset, m, dest 'q' or 'k', dest block)
PASS_K01 = (128, 128, "k", 0)
PASS_Q01 = (0, 128, "q", 0)
PASS_Q2 = (256, 64, "q", 1)
PASS_K2 = (320, 64, "k", 1)


def _body(nc, tc, xT, wqkT, wvT, woTp, bo_d, out_d):
    with (
        tc.tile_pool(name="const", bufs=1) as constp,
        tc.tile_pool(name="big", bufs=1) as bigp,
        tc.tile_pool(name="esp", bufs=PVLAG + 10) as esp,
        tc.tile_pool(name="smallp", bufs=4) as smallp,
        tc.tile_pool(name="ctxfp", bufs=2) as ctxfp,
        tc.tile_pool(name="outp", bufs=2) as outp,
        tc.tile_pool(name="psS", bufs=2, space="PSUM") as psS,
        tc.tile_pool(name="psC", bufs=3, space="PSUM") as psC,
        tc.tile_pool(name="psM", bufs=1, space="PSUM") as psM,
        tc.tile_pool(name="dram", bufs=1, space="DRAM") as dramp,
    ):
        # PSUM budget (8 banks): psS 2x[128,1024] (4) scores + lead-in
        # projections; psC 3x[65,512] (3) PV accumulators (2 live per
        # q-half) + reciprocal-broadcast tiles; psM 1x[128,512] (1) for the
        # interleaved projection / output-projection groups.  All psM/psC
        # tiles are allocated in CONSUMPTION order (lazily where needed) so
        # the pool ring dependencies match the emission order.

        # ---- load inputs to SBUF (all bf16 except bias) ----
        xT_sb = [bigp.tile([P, N], bf16, name=f"xT_sb_{cb}") for cb in range(CB)]
        wqkT_sb = bigp.tile([P, CB * 384], bf16, name="wqkT_sb")
        wvT_sb = bigp.tile([P, CB * 192], bf16, name="wvT_sb")
        woTp_sb = bigp.tile([P, CB * C], bf16, name="woTp_sb")
        bo_sb = bigp.tile([P, CB], f32, name="bo_sb")
        ones_sb = constp.tile([P, 64], bf16, name="ones_sb")
        nc.vector.memset(ones_sb[:, :], 1.0)
        # selector for the reciprocal broadcast matmul: cols [qc*64, qc*64+64)
        # form a [2,64] tile whose row qc is ones.  Engines cannot address
        # partition 1 alone, so row 1 is filled via sbuf->sbuf DMA.
        selsrc = constp.tile([1, 128], bf16, name="selsrc")
        nc.vector.memset(selsrc[0:1, 0:64], 1.0)
        nc.vector.memset(selsrc[0:1, 64:128], 0.0)
        sel_sb = constp.tile([2, 128], bf16, name="sel_sb")
        nc.sync.dma_start(sel_sb[0:1, :], selsrc[0:1, :])
        nc.sync.dma_start(sel_sb[1:2, 0:64], selsrc[0:1, 64:128])
        nc.sync.dma_start(sel_sb[1:2, 64:128], selsrc[0:1, 0:64])
        zero_sb = constp.tile([P, QCH], f32, name="zero_sb")
        nc.vector.memset(zero_sb[:, :], 0.0)
        # warm the ACT ln+exp table set (one-time PSEUDO_LOAD) during loads
        warm_sb = constp.tile([P, 2], f32, name="warm_sb")
        nc.scalar.activation(warm_sb[0:1, 0:1], ones_sb[0:1, 0:1], Exp, scale=SCALE)
        nc.scalar.activation(warm_sb[0:1, 1:2], ones_sb[0:1, 0:1], Log)

        # loads split over three DGE queues; x gates the whole lead-in so
        # it is spread over all three.
        for cb in range(CB):
            nc.scalar.dma_start(wqkT_sb[:, cb * 384:(cb + 1) * 384], wqkT[cb * P:(cb + 1) * P, :])
        for cb in range(CB):
            nc.scalar.dma_start(bo_sb[:, cb:cb + 1], bo_d[cb * P:(cb + 1) * P, :])
        for cb in range(CB):
            eng = nc.sync if cb % 2 == 0 else nc.gpsimd
            eng.dma_start(xT_sb[cb][:, :], xT[cb * P:(cb + 1) * P, :])
            nc.scalar.dma_start(wvT_sb[:, cb * 192:(cb + 1) * 192], wvT[cb * P:(cb + 1) * P, :])
        for cb in range(CB):
            nc.scalar.dma_start(woTp_sb[:, cb * C:(cb + 1) * C], woTp[cb * P:(cb + 1) * P, :])

        # bias broadcast [128, 6*512] f32 built once while ACT is idle; it
        # seeds the output-projection accumulator chain.
        bias_bc = bigp.tile([P, CB * QCH], f32, name="bias_bc")
        for cbo in range(CB):
            nc.scalar.activation(bias_bc[:, cbo * QCH:(cbo + 1) * QCH],
                                 zero_sb[:, :], Identity, bias=bo_sb[:, cbo:cbo + 1])

        qT_sb = bigp.tile([P, 2 * N], bf16, name="qT_sb")
        kT_sb = bigp.tile([P, 2 * N], bf16, name="kT_sb")
        v_sb = bigp.tile([P, KB * VW], bf16, name="v_sb")
        ctxT_sb = bigp.tile([64, HPC * N], bf16, name="ctxT_sb")
        accA = bigp.tile([P, CB * QCH], f32, name="accA")
        accB = bigp.tile([P, CB * QCH], f32, name="accB")
        dtmp_sb = bigp.tile([65, 1024], f32, name="dtmp_sb")

        # ---- emission helpers ------------------------------------------
        def qk_group_psS(co, m, dst, blk, qp):
            """Lead-in only: one [m, 1024] projection group on the psS ring
            (12 matmuls + DVE drain), emitted immediately."""
            dst_sb = qT_sb if dst == "q" else kT_sb
            ps = psS.tile([P, 2 * QCH], f32, name=f"pj_{dst}_{blk}_{qp}", tag="psS")
            for half in range(2):
                qn = qp * 2 + half
                for cb in range(CB):
                    nc.tensor.matmul(
                        ps[:m, half * QCH:(half + 1) * QCH],
                        lhsT=wqkT_sb[:, cb * 384 + co: cb * 384 + co + m],
                        rhs=xT_sb[cb][:, qn * QCH: qn * QCH + QCH],
                        start=(cb == 0), stop=(cb == CB - 1),
                    )
            nc.vector.tensor_copy(
                dst_sb[:m, blk * N + qp * 2 * QCH: blk * N + (qp + 1) * 2 * QCH],
                ps[:m, :])

        def qk_group512(co, m, dst, blk, qn):
            """Deferred [m, 512] projection group on psM: 3 closures."""
            dst_sb = qT_sb if dst == "q" else kT_sb
            cell = []

            def mm0():
                cell.append(psM.tile([P, QCH], f32, name=f"pj2_{dst}_{blk}_{qn}", tag="psM"))
                for cb in range(3):
                    nc.tensor.matmul(
                        cell[0][:m, 0:QCH],
                        lhsT=wqkT_sb[:, cb * 384 + co: cb * 384 + co + m],
                        rhs=xT_sb[cb][:, qn * QCH: qn * QCH + QCH],
                        start=(cb == 0), stop=False,
                    )

            def mm1():
                for cb in range(3, CB):
                    nc.tensor.matmul(
                        cell[0][:m, 0:QCH],
                        lhsT=wqkT_sb[:, cb * 384 + co: cb * 384 + co + m],
                        rhs=xT_sb[cb][:, qn * QCH: qn * QCH + QCH],
                        start=False, stop=(cb == CB - 1),
                    )

            def drain():
                nc.vector.tensor_copy(
                    dst_sb[:m, blk * N + qn * QCH: blk * N + (qn + 1) * QCH],
                    cell[0][:m, 0:QCH])
            return [mm0, mm1, drain]

        def v_group(nb):
            ps = psM.tile([P, QCH], f32, name=f"vps_{nb}", tag="psM")
            for cb in range(CB):
                nc.tensor.matmul(
                    ps[:, 0:192],
                    lhsT=xT_sb[cb][:, nb * P:(nb + 1) * P],
                    rhs=wvT_sb[:, cb * 192:(cb + 1) * 192],
                    start=(cb == 0), stop=(cb == CB - 1),
                )
            vv = v_sb[:, nb * VW:(nb + 1) * VW].rearrange("p (h w) -> p h w", h=HPC)
            pp = ps[:, 0:192].rearrange("p (h w) -> p h w", h=HPC)
            nc.vector.tensor_copy(vv[:, :, 0:64], pp[:, :, :])
            nc.vector.memset(vv[:, :, 64:65], 1.0)

        # ---- lead-in: ALL projections (Q/K both blocks, V).  Keeping the
        # lead-in dense lets the tensor engine ramp to its max p-state and
        # leaves the attention loop strictly ACT-bound (in-loop PE work per
        # iteration stays under the ~1.15us exp period).
        co, m, dst, blk = PASS_K01
        for qp in range(2):
            qk_group_psS(co, m, dst, blk, qp)
        co, m, dst, blk = PASS_Q01
        qk_group_psS(co, m, dst, blk, 0)
        for nb in range(8):
            v_group(nb)
        co, m, dst, blk = PASS_Q01
        qk_group_psS(co, m, dst, blk, 1)
        for nb in range(8, KB):
            v_group(nb)
        for co, m, dst, blk in (PASS_Q2, PASS_K2):
            for qp in range(2):
                qk_group_psS(co, m, dst, blk, qp)

        # only the per-round output projections remain as in-loop extras
        extras = []

        front = []

        def consume():
            if front:
                front.pop(0)()
            elif extras:
                extras.pop(0)()

        # ---- attention helpers -----------------------------------------
        def make_pv(h, cps, kb, es):
            def run():
                for qx in range(2):
                    nc.tensor.matmul(
                        cps[qx][0:65, :],
                        lhsT=v_sb[:, kb * VW + h * 65: kb * VW + (h + 1) * 65],
                        rhs=es[:, qx * QCH:(qx + 1) * QCH],
                        start=(kb == 0), stop=(kb == KB - 1),
                    )
            return run

        def norm_closures(h, f, cps):
            """Normalize the finished half into ctxT_sb.  bps tiles are
            allocated eagerly so the psC ring order (cps0, cps1, bps0, bps1
            per half) matches consumption order."""
            rec_in = smallp.tile([2, QCH], f32, name=f"rin_{h}_{f}", tag="rin")
            lt = smallp.tile([2, QCH], f32, name=f"lt_{h}_{f}", tag="lt")
            rec = smallp.tile([2, QCH], bf16, name=f"rec_{h}_{f}", tag="rec")
            rb = [smallp.tile([64, QCH], bf16, name=f"rb{qx}_{h}_{f}", tag=f"rb{qx}")
                  for qx in range(2)]
            bps0 = psC.tile([65, QCH], f32, name=f"bps0_{h}_{f}", tag="psC")
            bps1 = psC.tile([65, QCH], f32, name=f"bps1_{h}_{f}", tag="psC")

            def denoms():
                for qx in range(2):
                    nc.vector.tensor_copy(
                        dtmp_sb[64:65, qx * QCH:(qx + 1) * QCH],
                        cps[qx][64:65, :])
                nc.sync.dma_start(rec_in[0:1, :], dtmp_sb[64:65, 0:QCH])
                nc.sync.dma_start(rec_in[1:2, :], dtmp_sb[64:65, QCH:2 * QCH])

            def lnexp():
                nc.scalar.activation(lt, rec_in, Log)
                nc.scalar.activation(rec, lt, Exp, scale=-1.0)
                nc.tensor.matmul(bps0[0:64, :], lhsT=sel_sb[:, 0:64], rhs=rec[:, :],
                                 start=True, stop=True)

            def mul0():
                qc = 2 * f
                nc.vector.tensor_copy(rb[0][:, :], bps0[0:64, :])
                nc.tensor.matmul(bps1[0:64, :], lhsT=sel_sb[:, 64:128], rhs=rec[:, :],
                                 start=True, stop=True)
                nc.vector.tensor_mul(
                    ctxT_sb[0:64, h * N + qc * QCH: h * N + (qc + 1) * QCH],
                    cps[0][0:64, :], rb[0][:, :])

            def mul1():
                qc = 2 * f + 1
                nc.vector.tensor_copy(rb[1][:, :], bps1[0:64, :])
                nc.vector.tensor_mul(
                    ctxT_sb[0:64, h * N + qc * QCH: h * N + (qc + 1) * QCH],
                    cps[1][0:64, :], rb[1][:, :])
                # the previous half's psC slots are now fully released:
                # PV pops for the new half may resume (see boundary_ok)
                boundary_ok[0] = True
            return [denoms, lnexp, mul0, mul1]

        def a2a_stage(h):
            # ctxf row = s4*32 + dl, block dh at cols dh*512 (dh-major so
            # the last head's A2A can be split into two d-row pieces).
            # Recvs ride the gpsimd queue -- on sync they head-of-line
            # block the next boundary's DMAs behind the collective sem.
            send_h = dramp.tile([NCORES, 64, RQ], bf16, name=f"send_{h}")
            recv_h = dramp.tile([NCORES, 64, RQ], bf16, name=f"recv_{h}")
            ctxf = ctxfp.tile([P, 2 * QCH], bf16, name=f"ctxf_{h}", tag="ctxf")

            def stage():
                for j in range(NCORES):
                    nc.sync.dma_start(send_h[j, :, :],
                                      ctxT_sb[:, h * N + j * RQ: h * N + (j + 1) * RQ])
                nc.gpsimd.collective_compute(
                    "AllToAll", mybir.AluOpType.bypass,
                    replica_groups=[list(range(NCORES))],
                    ins=[send_h.opt()], outs=[recv_h.opt()],
                )
                for s in range(NCORES):
                    s4 = s % 4
                    for dh in range(2):
                        nc.gpsimd.dma_start(
                            ctxf[s4 * 32: s4 * 32 + 32,
                                 dh * QCH + (s // 4) * RQ: dh * QCH + (s // 4) * RQ + RQ],
                            recv_h[s, dh * 32:(dh + 1) * 32, :])
            return stage, ctxf

        def outproj_closures(r, ctxf, src_acc, dst_acc):
            cls = []
            for cbo in range(CB):
                def run(cbo=cbo):
                    ps = psM.tile([P, QCH], f32, name=f"ops_{r}_{cbo}", tag="psM")
                    for jb in range(2):
                        nc.tensor.matmul(
                            ps[:, :],
                            lhsT=woTp_sb[:, (2 * r + jb) * C + cbo * P: (2 * r + jb) * C + (cbo + 1) * P],
                            rhs=ctxf[:, jb * QCH:(jb + 1) * QCH],
                            start=(jb == 0), stop=(jb == 1),
                        )
                    nc.vector.tensor_add(
                        dst_acc[:, cbo * QCH:(cbo + 1) * QCH],
                        ps[:, :], src_acc[:, cbo * QCH:(cbo + 1) * QCH])
                cls.append(run)
            return cls

        # ---- main attention loop ---------------------------------------
        acc_chain = [(bias_bc, accB), (accB, accA)]
        pending_a2a = []  # (round, ctxf, release_iter)
        iter_no = 0
        last_cps = None
        # PV matmuls for a new half may only be emitted once the previous
        # half's normalization (which releases the psC ring slots via PE
        # broadcast matmuls emitted in the norm closures) is in the stream;
        # otherwise the in-order PE queue deadlocks on the slot wait.
        boundary_ok = [True]

        for h in range(HPC):
            hb_, ho_ = HOFF[h]
            for f in range(2):
                cps = [psC.tile([65, QCH], f32, name=f"cps_{h}_{f}_{qx}", tag="psC")
                       for qx in range(2)]
                pvq = []
                for kb in range(KB):
                    sps = psS.tile([P, 2 * QCH], f32, name=f"sps_{h}_{f}_{kb}", tag="psS")
                    for qx in range(2):
                        qc = 2 * f + qx
                        nc.tensor.matmul(
                            sps[:, qx * QCH:(qx + 1) * QCH],
                            lhsT=kT_sb[ho_:ho_ + 64, hb_ * N + kb * P: hb_ * N + (kb + 1) * P],
                            rhs=qT_sb[ho_:ho_ + 64, hb_ * N + qc * QCH: hb_ * N + qc * QCH + QCH],
                            start=True, stop=True,
                        )
                    es = esp.tile([P, 2 * QCH], bf16, name=f"es_{h}_{f}_{kb}", tag="es")
                    nc.scalar.activation(es, sps, Exp, scale=SCALE)
                    pvq.append(make_pv(h, cps, kb, es))
                    if pending_a2a and iter_no >= pending_a2a[0][2]:
                        r, ctxf, _ = pending_a2a.pop(0)
                        src, dst = acc_chain[r]
                        front.extend(outproj_closures(r, ctxf, src, dst))
                    consume()
                    if len(pvq) > PVLAG and boundary_ok[0]:
                        pvq.pop(0)()
                    iter_no += 1
                # end of half: flush the pending PVs two per slot, then
                # normalization (+ A2A staging when the head is done).
                for i in range(0, len(pvq), 2):
                    pair = pvq[i:i + 2]
                    front.append(lambda pair=pair: [c() for c in pair])
                pvq = []
                if h == HPC - 1 and f == 1:
                    last_cps = cps  # tail handles the final norm + A2A
                else:
                    boundary_ok[0] = False
                    front.extend(norm_closures(h, f, cps))
                    if f == 1:
                        stage, ctxf = a2a_stage(h)
                        front.append(stage)
                        pending_a2a.append((h, ctxf, iter_no + OUTPROJ_DELAY[h]))

        # ---- tail: final norm split by d-half, A2A in two pipelined
        # pieces, sends/recvs/stores spread over the idle queues ----------
        while front or extras:
            consume()
        r = HPC - 1
        cps = last_cps
        rec_in = smallp.tile([2, QCH], f32, name="rin_T", tag="rin")
        lt = smallp.tile([2, QCH], f32, name="lt_T", tag="lt")
        rec = smallp.tile([2, QCH], bf16, name="rec_T", tag="rec")
        rb = [smallp.tile([64, QCH], bf16, name=f"rb{qx}_T", tag=f"rb{qx}")
              for qx in range(2)]
        bps0 = psC.tile([65, QCH], f32, name="bps0_T", tag="psC")
        bps1 = psC.tile([65, QCH], f32, name="bps1_T", tag="psC")
        for qx in range(2):
            nc.vector.tensor_copy(dtmp_sb[64:65, qx * QCH:(qx + 1) * QCH],
                                  cps[qx][64:65, :])
        nc.sync.dma_start(rec_in[0:1, :], dtmp_sb[64:65, 0:QCH])
        nc.sync.dma_start(rec_in[1:2, :], dtmp_sb[64:65, QCH:2 * QCH])
        nc.scalar.activation(lt, rec_in, Log)
        nc.scalar.activation(rec, lt, Exp, scale=-1.0)
        nc.tensor.matmul(bps0[0:64, :], lhsT=sel_sb[:, 0:64], rhs=rec[:, :],
                         start=True, stop=True)
        nc.vector.tensor_copy(rb[0][:, :], bps0[0:64, :])
        nc.tensor.matmul(bps1[0:64, :], lhsT=sel_sb[:, 64:128], rhs=rec[:, :],
                         start=True, stop=True)
        nc.vector.tensor_copy(rb[1][:, :], bps1[0:64, :])

        ctxf_t = ctxfp.tile([P, 2 * QCH], bf16, name="ctxf_T", tag="ctxf")
        send_p = [dramp.tile([NCORES, 32, RQ], bf16, name=f"sendT_{dh}") for dh in range(2)]
        recv_p = [dramp.tile([NCORES, 32, RQ], bf16, name=f"recvT_{dh}") for dh in range(2)]
        for dh in range(2):
            # normalize only this d-half's 32 rows, stage, and trigger --
            # the lo piece's collective starts while the hi rows multiply
            for qx in range(2):
                qc = 2 + qx
                nc.vector.tensor_mul(
                    ctxT_sb[dh * 32:(dh + 1) * 32, r * N + qc * QCH: r * N + (qc + 1) * QCH],
                    cps[qx][dh * 32:(dh + 1) * 32, :], rb[qx][dh * 32:(dh + 1) * 32, :])
            for j in range(NCORES):
                eng = nc.sync if j % 2 == 0 else nc.scalar
                eng.dma_start(send_p[dh][j, :, :],
                              ctxT_sb[dh * 32:(dh + 1) * 32, r * N + j * RQ: r * N + (j + 1) * RQ])
            nc.gpsimd.collective_compute(
                "AllToAll", mybir.AluOpType.bypass,
                replica_groups=[list(range(NCORES))],
                ins=[send_p[dh].opt()], outs=[recv_p[dh].opt()],
            )
        for dh in range(2):
            engs = [nc.gpsimd, nc.scalar, nc.sync]
            for s in range(NCORES):
                s4 = s % 4
                engs[s % 3].dma_start(
                    ctxf_t[s4 * 32: s4 * 32 + 32,
                           dh * QCH + (s // 4) * RQ: dh * QCH + (s // 4) * RQ + RQ],
                    recv_p[dh][s, :, :])
            src = accA if dh == 0 else accB
            for cbo in range(CB):
                ps = psS.tile([P, 2 * QCH], f32, name=f"opsT_{dh}_{cbo}", tag="psS")
                nc.tensor.matmul(
                    ps[:, 0:QCH],
                    lhsT=woTp_sb[:, (2 * r + dh) * C + cbo * P: (2 * r + dh) * C + (cbo + 1) * P],
                    rhs=ctxf_t[:, dh * QCH:(dh + 1) * QCH],
                    start=True, stop=True,
                )
                if dh == 0:
                    nc.vector.tensor_add(
                        accB[:, cbo * QCH:(cbo + 1) * QCH],
                        ps[:, 0:QCH], src[:, cbo * QCH:(cbo + 1) * QCH])
                else:
                    osb = outp.tile([P, QCH], f32, name=f"osb_{cbo}", tag="osb")
                    nc.vector.tensor_add(osb, ps[:, 0:QCH], src[:, cbo * QCH:(cbo + 1) * QCH])
                    eng = nc.sync if cbo % 2 == 0 else nc.scalar
                    eng.dma_start(out_d[cbo * P:(cbo + 1) * P, :], osb)


_JOINT_SET = "natural_log_exp_and_others"


def _pin_act_tables():
    """Restrict the activation-table chooser to the one set containing
    exp+ln+identity, so the compiled stream has a single ACT_TABLE_LOAD
    instead of flip-flopping between exp_and_others and natural_log (a
    ~1.3us stall at every half boundary).  Only the table *selection* for
    this kernel's own compile is affected; list positions (the
    act_func_set_id encoding) are preserved."""
    orig = bacc.get_activation_tables

    def pinned(arch):
        t = orig(arch)
        if _JOINT_SET not in t:
            return t
        return {k: (v if k == _JOINT_SET else set()) for k, v in t.items()}

    bacc.get_activation_tables = pinned
    return orig


def build():
    orig_tables = _pin_act_tables()
    try:
        return _build()
    finally:
        bacc.get_activation_tables = orig_tables


def _build():
    nc = bacc.Bacc("TRN2", target_bir_lowering=False, debug=False, num_devices=NCORES)
    xT = nc.dram_tensor("xT", [C, N], bf16, kind="ExternalInput").ap()
    wqkT = nc.dram_tensor("wqkT", [C, 2 * HPC * HD], bf16, kind="ExternalInput").ap()
    wvT = nc.dram_tensor("wvT", [C, HPC * HD], bf16, kind="ExternalInput").ap()
    woTp = nc.dram_tensor("woTp", [C, C], bf16, kind="ExternalInput").ap()
    bo_d = nc.dram_tensor("bo", [C, 1], f32, kind="ExternalInput").ap()
    out_d = nc.dram_tensor("out", [C, 2 * RQ], f32, kind="ExternalOutput").ap()
    with tile.TileContext(nc) as tc:
        _body(nc, tc, xT, wqkT, wvT, woTp, bo_d, out_d)
    nc.compile()
    return nc


_NC = None


def _get_nc():
    global _NC
    if _NC is None:
        _NC = build()
    return _NC


# Wo row permutation (dh-major): A2A round r, d-half dh delivers global
# heads {s4*3+r} rows [dh*32,(dh+1)*32) as contiguous channel block
# r*256 + dh*128 + s4*32 + dl.
_PERM = np.array([(s4 * 3 + r) * 64 + dh * 32 + dl
                  for r in range(HPC) for dh in range(2)
                  for s4 in range(4) for dl in range(32)])


def make_in_maps(x, Wq, Wk, Wv, Wo, bo):
    x = np.asarray(x, np.float32)
    woTp = np.ascontiguousarray(np.asarray(Wo, np.float32).T[_PERM, :]).astype(ml_dtypes.bfloat16)
    bo_col = np.ascontiguousarray(np.asarray(bo, np.float32).reshape(C, 1))
    in_maps = []
    for i in range(NCORES):
        b = i // 4
        hs = (i % 4) * HPC
        rq = slice(hs * HD, (hs + HPC) * HD)
        wq_s = np.asarray(Wq, np.float32)[rq]  # [192, 768]
        wk_s = np.asarray(Wk, np.float32)[rq]
        # column order: [q0 q1 | k0 k1 | q2 | k2]
        wqk = np.concatenate([wq_s[0:128], wk_s[0:128], wq_s[128:192], wk_s[128:192]], axis=0).T
        in_maps.append({
            "xT": np.ascontiguousarray(x[b].T).astype(ml_dtypes.bfloat16),
            "wqkT": np.ascontiguousarray(wqk).astype(ml_dtypes.bfloat16),
            "wvT": np.ascontiguousarray(np.asarray(Wv, np.float32)[rq].T).astype(ml_dtypes.bfloat16),
            "woTp": woTp,
            "bo": bo_col,
        })
    return in_maps


def unshard(results):
    out = np.empty((B, N, C), np.float32)
    for i, r in enumerate(results):
        o = r["out"]  # [768, 512]: cols 0-255 batch 0, 256-511 batch 1
        out[0, i * RQ:(i + 1) * RQ, :] = o[:, :RQ].T
        out[1, i * RQ:(i + 1) * RQ, :] = o[:, RQ:].T
    return out


def kernel(x, Wq, Wk, Wv, Wo, bo):
    nc = _get_nc()
    in_maps = make_in_maps(x, Wq, Wk, Wv, Wo, bo)
    res = run_bass_kernel_spmd(nc, in_maps, core_ids=list(range(NCORES)))
    return unshard(res.results)


# revision 36
# speedup vs baseline: 1.0252x; 1.0252x over previous
"""Distributed multi-head attention kernel for 8 TRN2 NeuronCores.

Problem: B=2, N=2048, C=768, H=12 heads of dim 64.
  q = x @ Wq.T ; k = x @ Wk.T ; v = x @ Wv.T      (per-head split)
  out = softmax(q k^T / 8) v                        (full N^2 attention)
  y = concat_heads(out) @ Wo.T + bo

Sharding: 24 (batch, head) pairs -> 3 per core.  Core i owns batch i//4 and
heads 3*(i%4)..3*(i%4)+2.  Projections + attention are fully local (weights
row-sliced on the host).  A per-head 8-way AllToAll redistributes the
context so core i owns query rows 256*i..256*(i+1) of BOTH batches, and the
output projection accumulates PER A2A ROUND (4 global heads at a time, Wo
row-permuted dh-major on the host), leaving only the last head's A2A on the
critical path -- and that one is split into two d-half pieces pipelined
against its own output projection.

Schedule: the attention inner loop is ACT(exp)-bound (~1.15us per key-block
covering 2 q-chunks, [128,1024] exp tiles on a 2-deep PSUM ring).  All
other PE work (later heads' Q/K projections, per-round output projection,
reciprocal broadcasts) is interleaved one small "extra" per iteration so
the PE stream stays dense without out-running the ACT.  Softmax
normalization computes 1/d = exp(-ln(d)) on the ACT (ln+exp+identity share
one activation table set, pinned at compile time so walrus emits a single
ACT_TABLE_LOAD) and broadcasts the reciprocal row across partitions with a
tiny [2,64]-selector matmul -- no DRAM round-trip, no slow DVE reciprocal.
"""

import numpy as np
import ml_dtypes

import concourse.mybir as mybir
import concourse.tile as tile
from concourse import bacc
from concourse.bass_utils import run_bass_kernel_spmd

B, N, C, H, HD = 2, 2048, 768, 12, 64
SCALE = HD ** -0.5          # 0.125
P = 128
CB = C // P                 # 6 contraction blocks of 128 over channels
KB = N // P                 # 16 key blocks
QCH = 512                   # query chunk (max moving free dim)
HPC = 3                     # heads per core
NCORES = 8
VW = HPC * (HD + 1)         # 195: v columns per key-block (3 heads + ones col)
RQ = N // NCORES            # 256 query rows per core per batch after A2A
PVLAG = 6                   # PV trails scores by this many key-blocks
# iterations from A2A staging to its out-proj work (waits for the
# collective to land so the PE stream never head-of-line blocks on it)
OUTPROJ_DELAY = {0: 34, 1: 26}

f32 = mybir.dt.float32
bf16 = mybir.dt.bfloat16
Exp = mybir.ActivationFunctionType.Exp
Log = mybir.ActivationFunctionType.Ln
Identity = mybir.ActivationFunctionType.Identity

# head -> (block, partition offset) inside qT_sb / kT_sb [128, 2*2048].
HOFF = {0: (0, 0), 1: (0, 64), 2: (1, 0)}
# wqkT host column order: [q0 q1 | k0 k1 | q2 | k2]
# (col offset, m, dest 'q' or 'k', dest block)
PASS_K01 = (128, 128, "k", 0)
PASS_Q01 = (0, 128, "q", 0)
PASS_Q2 = (256, 64, "q", 1)
PASS_K2 = (320, 64, "k", 1)


def _body(nc, tc, xT, wqkT, wvT, woTp, bo_d, out_d):
    with (
        tc.tile_pool(name="const", bufs=1) as constp,
        tc.tile_pool(name="big", bufs=1) as bigp,
        tc.tile_pool(name="esp", bufs=PVLAG + 10) as esp,
        tc.tile_pool(name="smallp", bufs=4) as smallp,
        tc.tile_pool(name="ctxfp", bufs=2) as ctxfp,
        tc.tile_pool(name="outp", bufs=2) as outp,
        tc.tile_pool(name="psS", bufs=2, space="PSUM") as psS,
        tc.tile_pool(name="psC", bufs=3, space="PSUM") as psC,
        tc.tile_pool(name="psM", bufs=1, space="PSUM") as psM,
        tc.tile_pool(name="dram", bufs=1, space="DRAM") as dramp,
    ):
        # PSUM budget (8 banks): psS 2x[128,1024] (4) scores + lead-in
        # projections; psC 3x[65,512] (3) PV accumulators (2 live per
        # q-half) + reciprocal-broadcast tiles; psM 1x[128,512] (1) for the
        # interleaved projection / output-projection groups.  All psM/psC
        # tiles are allocated in CONSUMPTION order (lazily where needed) so
        # the pool ring dependencies match the emission order.

        # ---- load inputs to SBUF (all bf16 except bias) ----
        xT_sb = [bigp.tile([P, N], bf16, name=f"xT_sb_{cb}") for cb in range(CB)]
        wqkT_sb = bigp.tile([P, CB * 384], bf16, name="wqkT_sb")
        wvT_sb = bigp.tile([P, CB * 192], bf16, name="wvT_sb")
        woTp_sb = bigp.tile([P, CB * C], bf16, name="woTp_sb")
        bo_sb = bigp.tile([P, CB], f32, name="bo_sb")
        ones_sb = constp.tile([P, 64], bf16, name="ones_sb")
        nc.vector.memset(ones_sb[:, :], 1.0)
        # selector for the reciprocal broadcast matmul: cols [qc*64, qc*64+64)
        # form a [2,64] tile whose row qc is ones.  Engines cannot address
        # partition 1 alone, so row 1 is filled via sbuf->sbuf DMA.
        selsrc = constp.tile([1, 128], bf16, name="selsrc")
        nc.vector.memset(selsrc[0:1, 0:64], 1.0)
        nc.vector.memset(selsrc[0:1, 64:128], 0.0)
        sel_sb = constp.tile([2, 128], bf16, name="sel_sb")
        nc.sync.dma_start(sel_sb[0:1, :], selsrc[0:1, :])
        nc.sync.dma_start(sel_sb[1:2, 0:64], selsrc[0:1, 64:128])
        nc.sync.dma_start(sel_sb[1:2, 64:128], selsrc[0:1, 0:64])
        zero_sb = constp.tile([P, QCH], f32, name="zero_sb")
        nc.vector.memset(zero_sb[:, :], 0.0)
        # warm the ACT ln+exp table set (one-time PSEUDO_LOAD) during loads
        warm_sb = constp.tile([P, 2], f32, name="warm_sb")
        nc.scalar.activation(warm_sb[0:1, 0:1], ones_sb[0:1, 0:1], Exp, scale=SCALE)
        nc.scalar.activation(warm_sb[0:1, 1:2], ones_sb[0:1, 0:1], Log)

        # loads split over three DGE queues; x gates the whole lead-in so
        # it is spread over all three.
        for cb in range(CB):
            nc.scalar.dma_start(wqkT_sb[:, cb * 384:(cb + 1) * 384], wqkT[cb * P:(cb + 1) * P, :])
        for cb in range(CB):
            nc.scalar.dma_start(bo_sb[:, cb:cb + 1], bo_d[cb * P:(cb + 1) * P, :])
        for cb in range(CB):
            eng = nc.sync if cb % 2 == 0 else nc.gpsimd
            eng.dma_start(xT_sb[cb][:, :], xT[cb * P:(cb + 1) * P, :])
            nc.scalar.dma_start(wvT_sb[:, cb * 192:(cb + 1) * 192], wvT[cb * P:(cb + 1) * P, :])
        for cb in range(CB):
            nc.scalar.dma_start(woTp_sb[:, cb * C:(cb + 1) * C], woTp[cb * P:(cb + 1) * P, :])

        # bias broadcast [128, 6*512] f32 built once while ACT is idle; it
        # seeds the output-projection accumulator chain.
        bias_bc = bigp.tile([P, CB * QCH], f32, name="bias_bc")
        for cbo in range(CB):
            nc.scalar.activation(bias_bc[:, cbo * QCH:(cbo + 1) * QCH],
                                 zero_sb[:, :], Identity, bias=bo_sb[:, cbo:cbo + 1])

        qT_sb = bigp.tile([P, 2 * N], bf16, name="qT_sb")
        kT_sb = bigp.tile([P, 2 * N], bf16, name="kT_sb")
        v_sb = bigp.tile([P, KB * VW], bf16, name="v_sb")
        ctxT_sb = bigp.tile([64, HPC * N], bf16, name="ctxT_sb")
        accA = bigp.tile([P, CB * QCH], f32, name="accA")
        accB = bigp.tile([P, CB * QCH], f32, name="accB")
        dtmp_sb = bigp.tile([65, 1024], f32, name="dtmp_sb")

        # ---- emission helpers ------------------------------------------
        def qk_group_psS(co, m, dst, blk, qp):
            """Lead-in only: one [m, 1024] projection group on the psS ring
            (12 matmuls + DVE drain), emitted immediately."""
            dst_sb = qT_sb if dst == "q" else kT_sb
            ps = psS.tile([P, 2 * QCH], f32, name=f"pj_{dst}_{blk}_{qp}", tag="psS")
            for half in range(2):
                qn = qp * 2 + half
                for cb in range(CB):
                    nc.tensor.matmul(
                        ps[:m, half * QCH:(half + 1) * QCH],
                        lhsT=wqkT_sb[:, cb * 384 + co: cb * 384 + co + m],
                        rhs=xT_sb[cb][:, qn * QCH: qn * QCH + QCH],
                        start=(cb == 0), stop=(cb == CB - 1),
                    )
            nc.vector.tensor_copy(
                dst_sb[:m, blk * N + qp * 2 * QCH: blk * N + (qp + 1) * 2 * QCH],
                ps[:m, :])

        def qk_group512(co, m, dst, blk, qn):
            """Deferred [m, 512] projection group on psM: 3 closures."""
            dst_sb = qT_sb if dst == "q" else kT_sb
            cell = []

            def mm0():
                cell.append(psM.tile([P, QCH], f32, name=f"pj2_{dst}_{blk}_{qn}", tag="psM"))
                for cb in range(3):
                    nc.tensor.matmul(
                        cell[0][:m, 0:QCH],
                        lhsT=wqkT_sb[:, cb * 384 + co: cb * 384 + co + m],
                        rhs=xT_sb[cb][:, qn * QCH: qn * QCH + QCH],
                        start=(cb == 0), stop=False,
                    )

            def mm1():
                for cb in range(3, CB):
                    nc.tensor.matmul(
                        cell[0][:m, 0:QCH],
                        lhsT=wqkT_sb[:, cb * 384 + co: cb * 384 + co + m],
                        rhs=xT_sb[cb][:, qn * QCH: qn * QCH + QCH],
                        start=False, stop=(cb == CB - 1),
                    )

            def drain():
                nc.vector.tensor_copy(
                    dst_sb[:m, blk * N + qn * QCH: blk * N + (qn + 1) * QCH],
                    cell[0][:m, 0:QCH])
            return [mm0, mm1, drain]

        def v_group(nb):
            ps = psM.tile([P, QCH], f32, name=f"vps_{nb}", tag="psM")
            for cb in range(CB):
                nc.tensor.matmul(
                    ps[:, 0:192],
                    lhsT=xT_sb[cb][:, nb * P:(nb + 1) * P],
                    rhs=wvT_sb[:, cb * 192:(cb + 1) * 192],
                    start=(cb == 0), stop=(cb == CB - 1),
                )
            vv = v_sb[:, nb * VW:(nb + 1) * VW].rearrange("p (h w) -> p h w", h=HPC)
            pp = ps[:, 0:192].rearrange("p (h w) -> p h w", h=HPC)
            nc.vector.tensor_copy(vv[:, :, 0:64], pp[:, :, :])
            nc.vector.memset(vv[:, :, 64:65], 1.0)

        # ---- lead-in: k0/k1 (all kb), q0/q1 chunks 0-1, all of V --------
        co, m, dst, blk = PASS_K01
        for qp in range(2):
            qk_group_psS(co, m, dst, blk, qp)
        co, m, dst, blk = PASS_Q01
        qk_group_psS(co, m, dst, blk, 0)
        for nb in range(KB):
            v_group(nb)

        # ---- deferred extras (consumed one per attention iteration);
        # ordered by deadline: q0/q1 chunks 2-3 by iter 16, q2/k2 by 64.
        extras = []
        co, m, dst, blk = PASS_Q01
        for qn in (2, 3):
            extras.extend(qk_group512(co, m, dst, blk, qn))
        for co, m, dst, blk in (PASS_Q2, PASS_K2):
            for qn in range(4):
                extras.extend(qk_group512(co, m, dst, blk, qn))

        front = []

        def consume():
            if front:
                front.pop(0)()
            elif extras:
                extras.pop(0)()

        # ---- attention helpers -----------------------------------------
        def make_pv(h, cps, kb, es):
            def run():
                for qx in range(2):
                    nc.tensor.matmul(
                        cps[qx][0:65, :],
                        lhsT=v_sb[:, kb * VW + h * 65: kb * VW + (h + 1) * 65],
                        rhs=es[:, qx * QCH:(qx + 1) * QCH],
                        start=(kb == 0), stop=(kb == KB - 1),
                    )
            return run

        def norm_closures(h, f, cps):
            """Normalize the finished half into ctxT_sb.  bps tiles are
            allocated eagerly so the psC ring order (cps0, cps1, bps0, bps1
            per half) matches consumption order."""
            rec_in = smallp.tile([2, QCH], f32, name=f"rin_{h}_{f}", tag="rin")
            lt = smallp.tile([2, QCH], f32, name=f"lt_{h}_{f}", tag="lt")
            rec = smallp.tile([2, QCH], bf16, name=f"rec_{h}_{f}", tag="rec")
            rb = [smallp.tile([64, QCH], bf16, name=f"rb{qx}_{h}_{f}", tag=f"rb{qx}")
                  for qx in range(2)]
            bps0 = psC.tile([65, QCH], f32, name=f"bps0_{h}_{f}", tag="psC")
            bps1 = psC.tile([65, QCH], f32, name=f"bps1_{h}_{f}", tag="psC")

            def denoms():
                for qx in range(2):
                    nc.vector.tensor_copy(
                        dtmp_sb[64:65, qx * QCH:(qx + 1) * QCH],
                        cps[qx][64:65, :])
                nc.sync.dma_start(rec_in[0:1, :], dtmp_sb[64:65, 0:QCH])
                nc.sync.dma_start(rec_in[1:2, :], dtmp_sb[64:65, QCH:2 * QCH])

            def lnexp():
                nc.scalar.activation(lt, rec_in, Log)
                nc.scalar.activation(rec, lt, Exp, scale=-1.0)
                nc.tensor.matmul(bps0[0:64, :], lhsT=sel_sb[:, 0:64], rhs=rec[:, :],
                                 start=True, stop=True)

            def mul0():
                qc = 2 * f
                nc.vector.tensor_copy(rb[0][:, :], bps0[0:64, :])
                nc.tensor.matmul(bps1[0:64, :], lhsT=sel_sb[:, 64:128], rhs=rec[:, :],
                                 start=True, stop=True)
                nc.vector.tensor_mul(
                    ctxT_sb[0:64, h * N + qc * QCH: h * N + (qc + 1) * QCH],
                    cps[0][0:64, :], rb[0][:, :])

            def mul1():
                qc = 2 * f + 1
                nc.vector.tensor_copy(rb[1][:, :], bps1[0:64, :])
                nc.vector.tensor_mul(
                    ctxT_sb[0:64, h * N + qc * QCH: h * N + (qc + 1) * QCH],
                    cps[1][0:64, :], rb[1][:, :])
                # the previous half's psC slots are now fully released:
                # PV pops for the new half may resume (see boundary_ok)
                boundary_ok[0] = True
            return [denoms, lnexp, mul0, mul1]

        def a2a_stage(h):
            # ctxf row = s4*32 + dl, block dh at cols dh*512 (dh-major so
            # the last head's A2A can be split into two d-row pieces).
            # Recvs ride the gpsimd queue -- on sync they head-of-line
            # block the next boundary's DMAs behind the collective sem.
            send_h = dramp.tile([NCORES, 64, RQ], bf16, name=f"send_{h}")
            recv_h = dramp.tile([NCORES, 64, RQ], bf16, name=f"recv_{h}")
            ctxf = ctxfp.tile([P, 2 * QCH], bf16, name=f"ctxf_{h}", tag="ctxf")

            def stage():
                for j in range(NCORES):
                    nc.sync.dma_start(send_h[j, :, :],
                                      ctxT_sb[:, h * N + j * RQ: h * N + (j + 1) * RQ])
                nc.gpsimd.collective_compute(
                    "AllToAll", mybir.AluOpType.bypass,
                    replica_groups=[list(range(NCORES))],
                    ins=[send_h.opt()], outs=[recv_h.opt()],
                )
                for s in range(NCORES):
                    s4 = s % 4
                    for dh in range(2):
                        nc.gpsimd.dma_start(
                            ctxf[s4 * 32: s4 * 32 + 32,
                                 dh * QCH + (s // 4) * RQ: dh * QCH + (s // 4) * RQ + RQ],
                            recv_h[s, dh * 32:(dh + 1) * 32, :])
            return stage, ctxf

        def outproj_closures(r, ctxf, src_acc, dst_acc):
            cls = []
            for cbo in range(CB):
                def run(cbo=cbo):
                    ps = psM.tile([P, QCH], f32, name=f"ops_{r}_{cbo}", tag="psM")
                    for jb in range(2):
                        nc.tensor.matmul(
                            ps[:, :],
                            lhsT=woTp_sb[:, (2 * r + jb) * C + cbo * P: (2 * r + jb) * C + (cbo + 1) * P],
                            rhs=ctxf[:, jb * QCH:(jb + 1) * QCH],
                            start=(jb == 0), stop=(jb == 1),
                        )
                    nc.vector.tensor_add(
                        dst_acc[:, cbo * QCH:(cbo + 1) * QCH],
                        ps[:, :], src_acc[:, cbo * QCH:(cbo + 1) * QCH])
                cls.append(run)
            return cls

        # ---- main attention loop ---------------------------------------
        acc_chain = [(bias_bc, accB), (accB, accA)]
        pending_a2a = []  # (round, ctxf, release_iter)
        iter_no = 0
        last_cps = None
        # PV matmuls for a new half may only be emitted once the previous
        # half's normalization (which releases the psC ring slots via PE
        # broadcast matmuls emitted in the norm closures) is in the stream;
        # otherwise the in-order PE queue deadlocks on the slot wait.
        boundary_ok = [True]

        for h in range(HPC):
            hb_, ho_ = HOFF[h]
            for f in range(2):
                cps = [psC.tile([65, QCH], f32, name=f"cps_{h}_{f}_{qx}", tag="psC")
                       for qx in range(2)]
                pvq = []
                for kb in range(KB):
                    sps = psS.tile([P, 2 * QCH], f32, name=f"sps_{h}_{f}_{kb}", tag="psS")
                    for qx in range(2):
                        qc = 2 * f + qx
                        nc.tensor.matmul(
                            sps[:, qx * QCH:(qx + 1) * QCH],
                            lhsT=kT_sb[ho_:ho_ + 64, hb_ * N + kb * P: hb_ * N + (kb + 1) * P],
                            rhs=qT_sb[ho_:ho_ + 64, hb_ * N + qc * QCH: hb_ * N + qc * QCH + QCH],
                            start=True, stop=True,
                        )
                    es = esp.tile([P, 2 * QCH], bf16, name=f"es_{h}_{f}_{kb}", tag="es")
                    nc.scalar.activation(es, sps, Exp, scale=SCALE)
                    pvq.append(make_pv(h, cps, kb, es))
                    if pending_a2a and iter_no >= pending_a2a[0][2]:
                        r, ctxf, _ = pending_a2a.pop(0)
                        src, dst = acc_chain[r]
                        front.extend(outproj_closures(r, ctxf, src, dst))
                    consume()
                    if len(pvq) > PVLAG and boundary_ok[0]:
                        pvq.pop(0)()
                    iter_no += 1
                # end of half: flush the pending PVs two per slot, then
                # normalization (+ A2A staging when the head is done).
                for i in range(0, len(pvq), 2):
                    pair = pvq[i:i + 2]
                    front.append(lambda pair=pair: [c() for c in pair])
                pvq = []
                if h == HPC - 1 and f == 1:
                    last_cps = cps  # tail handles the final norm + A2A
                else:
                    boundary_ok[0] = False
                    front.extend(norm_closures(h, f, cps))
                    if f == 1:
                        stage, ctxf = a2a_stage(h)
                        front.append(stage)
                        pending_a2a.append((h, ctxf, iter_no + OUTPROJ_DELAY[h]))

        # ---- tail: final norm split by d-half, A2A in two pipelined
        # pieces, sends/recvs/stores spread over the idle queues ----------
        while front or extras:
            consume()
        r = HPC - 1
        cps = last_cps
        rec_in = smallp.tile([2, QCH], f32, name="rin_T", tag="rin")
        lt = smallp.tile([2, QCH], f32, name="lt_T", tag="lt")
        rec = smallp.tile([2, QCH], bf16, name="rec_T", tag="rec")
        rb = [smallp.tile([64, QCH], bf16, name=f"rb{qx}_T", tag=f"rb{qx}")
              for qx in range(2)]
        bps0 = psC.tile([65, QCH], f32, name="bps0_T", tag="psC")
        bps1 = psC.tile([65, QCH], f32, name="bps1_T", tag="psC")
        for qx in range(2):
            nc.vector.tensor_copy(dtmp_sb[64:65, qx * QCH:(qx + 1) * QCH],
                                  cps[qx][64:65, :])
        nc.sync.dma_start(rec_in[0:1, :], dtmp_sb[64:65, 0:QCH])
        nc.sync.dma_start(rec_in[1:2, :], dtmp_sb[64:65, QCH:2 * QCH])
        nc.scalar.activation(lt, rec_in, Log)
        nc.scalar.activation(rec, lt, Exp, scale=-1.0)
        nc.tensor.matmul(bps0[0:64, :], lhsT=sel_sb[:, 0:64], rhs=rec[:, :],
                         start=True, stop=True)
        nc.vector.tensor_copy(rb[0][:, :], bps0[0:64, :])
        nc.tensor.matmul(bps1[0:64, :], lhsT=sel_sb[:, 64:128], rhs=rec[:, :],
                         start=True, stop=True)
        nc.vector.tensor_copy(rb[1][:, :], bps1[0:64, :])

        ctxf_t = ctxfp.tile([P, 2 * QCH], bf16, name="ctxf_T", tag="ctxf")
        send_p = [dramp.tile([NCORES, 32, RQ], bf16, name=f"sendT_{dh}") for dh in range(2)]
        recv_p = [dramp.tile([NCORES, 32, RQ], bf16, name=f"recvT_{dh}") for dh in range(2)]
        for dh in range(2):
            # normalize only this d-half's 32 rows, stage, and trigger --
            # the lo piece's collective starts while the hi rows multiply
            for qx in range(2):
                qc = 2 + qx
                nc.vector.tensor_mul(
                    ctxT_sb[dh * 32:(dh + 1) * 32, r * N + qc * QCH: r * N + (qc + 1) * QCH],
                    cps[qx][dh * 32:(dh + 1) * 32, :], rb[qx][dh * 32:(dh + 1) * 32, :])
            for j in range(NCORES):
                eng = nc.sync if j % 2 == 0 else nc.scalar
                eng.dma_start(send_p[dh][j, :, :],
                              ctxT_sb[dh * 32:(dh + 1) * 32, r * N + j * RQ: r * N + (j + 1) * RQ])
            nc.gpsimd.collective_compute(
                "AllToAll", mybir.AluOpType.bypass,
                replica_groups=[list(range(NCORES))],
                ins=[send_p[dh].opt()], outs=[recv_p[dh].opt()],
            )
        for dh in range(2):
            engs = [nc.gpsimd, nc.scalar, nc.sync]
            for s in range(NCORES):
                s4 = s % 4
                engs[s % 3].dma_start(
                    ctxf_t[s4 * 32: s4 * 32 + 32,
                           dh * QCH + (s // 4) * RQ: dh * QCH + (s // 4) * RQ + RQ],
                    recv_p[dh][s, :, :])
            src = accA if dh == 0 else accB
            for cbo in range(CB):
                ps = psS.tile([P, 2 * QCH], f32, name=f"opsT_{dh}_{cbo}", tag="psS")
                nc.tensor.matmul(
                    ps[:, 0:QCH],
                    lhsT=woTp_sb[:, (2 * r + dh) * C + cbo * P: (2 * r + dh) * C + (cbo + 1) * P],
                    rhs=ctxf_t[:, dh * QCH:(dh + 1) * QCH],
                    start=True, stop=True,
                )
                if dh == 0:
                    nc.vector.tensor_add(
                        accB[:, cbo * QCH:(cbo + 1) * QCH],
                        ps[:, 0:QCH], src[:, cbo * QCH:(cbo + 1) * QCH])
                else:
                    osb = outp.tile([P, QCH], f32, name=f"osb_{cbo}", tag="osb")
                    nc.vector.tensor_add(osb, ps[:, 0:QCH], src[:, cbo * QCH:(cbo + 1) * QCH])
                    eng = nc.sync if cbo % 2 == 0 else nc.scalar
                    eng.dma_start(out_d[cbo * P:(cbo + 1) * P, :], osb)


_JOINT_SET = "natural_log_exp_and_others"


def _pin_act_tables():
    """Restrict the activation-table chooser to the one set containing
    exp+ln+identity, so the compiled stream has a single ACT_TABLE_LOAD
    instead of flip-flopping between exp_and_others and natural_log (a
    ~1.3us stall at every half boundary).  Only the table *selection* for
    this kernel's own compile is affected; list positions (the
    act_func_set_id encoding) are preserved."""
    orig = bacc.get_activation_tables

    def pinned(arch):
        t = orig(arch)
        if _JOINT_SET not in t:
            return t
        return {k: (v if k == _JOINT_SET else set()) for k, v in t.items()}

    bacc.get_activation_tables = pinned
    return orig


def build():
    orig_tables = _pin_act_tables()
    try:
        return _build()
    finally:
        bacc.get_activation_tables = orig_tables


def _build():
    nc = bacc.Bacc("TRN2", target_bir_lowering=False, debug=False, num_devices=NCORES)
    xT = nc.dram_tensor("xT", [C, N], bf16, kind="ExternalInput").ap()
    wqkT = nc.dram_tensor("wqkT", [C, 2 * HPC * HD], bf16, kind="ExternalInput").ap()
    wvT = nc.dram_tensor("wvT", [C, HPC * HD], bf16, kind="ExternalInput").ap()
    woTp = nc.dram_tensor("woTp", [C, C], bf16, kind="ExternalInput").ap()
    bo_d = nc.dram_tensor("bo", [C, 1], f32, kind="ExternalInput").ap()
    out_d = nc.dram_tensor("out", [C, 2 * RQ], f32, kind="ExternalOutput").ap()
    with tile.TileContext(nc) as tc:
        _body(nc, tc, xT, wqkT, wvT, woTp, bo_d, out_d)
    nc.compile()
    return nc


_NC = None


def _get_nc():
    global _NC
    if _NC is None:
        _NC = build()
    return _NC


# Wo row permutation (dh-major): A2A round r, d-half dh delivers global
# heads {s4*3+r} rows [dh*32,(dh+1)*32) as contiguous channel block
# r*256 + dh*128 + s4*32 + dl.
_PERM = np.array([(s4 * 3 + r) * 64 + dh * 32 + dl
                  for r in range(HPC) for dh in range(2)
                  for s4 in range(4) for dl in range(32)])


def make_in_maps(x, Wq, Wk, Wv, Wo, bo):
    x = np.asarray(x, np.float32)
    woTp = np.ascontiguousarray(np.asarray(Wo, np.float32).T[_PERM, :]).astype(ml_dtypes.bfloat16)
    bo_col = np.ascontiguousarray(np.asarray(bo, np.float32).reshape(C, 1))
    in_maps = []
    for i in range(NCORES):
        b = i // 4
        hs = (i % 4) * HPC
        rq = slice(hs * HD, (hs + HPC) * HD)
        wq_s = np.asarray(Wq, np.float32)[rq]  # [192, 768]
        wk_s = np.asarray(Wk, np.float32)[rq]
        # column order: [q0 q1 | k0 k1 | q2 | k2]
        wqk = np.concatenate([wq_s[0:128], wk_s[0:128], wq_s[128:192], wk_s[128:192]], axis=0).T
        in_maps.append({
            "xT": np.ascontiguousarray(x[b].T).astype(ml_dtypes.bfloat16),
            "wqkT": np.ascontiguousarray(wqk).astype(ml_dtypes.bfloat16),
            "wvT": np.ascontiguousarray(np.asarray(Wv, np.float32)[rq].T).astype(ml_dtypes.bfloat16),
            "woTp": woTp,
            "bo": bo_col,
        })
    return in_maps


def unshard(results):
    out = np.empty((B, N, C), np.float32)
    for i, r in enumerate(results):
        o = r["out"]  # [768, 512]: cols 0-255 batch 0, 256-511 batch 1
        out[0, i * RQ:(i + 1) * RQ, :] = o[:, :RQ].T
        out[1, i * RQ:(i + 1) * RQ, :] = o[:, RQ:].T
    return out


def kernel(x, Wq, Wk, Wv, Wo, bo):
    nc = _get_nc()
    in_maps = make_in_maps(x, Wq, Wk, Wv, Wo, bo)
    res = run_bass_kernel_spmd(nc, in_maps, core_ids=list(range(NCORES)))
    return unshard(res.results)


# revision 37
# speedup vs baseline: 1.0761x; 1.0496x over previous
"""Distributed multi-head attention kernel for 8 TRN2 NeuronCores.

Problem: B=2, N=2048, C=768, H=12 heads of dim 64.
  q = x @ Wq.T ; k = x @ Wk.T ; v = x @ Wv.T      (per-head split)
  out = softmax(q k^T / 8) v                        (full N^2 attention)
  y = concat_heads(out) @ Wo.T + bo

Sharding: 24 (batch, head) pairs -> 3 per core.  Core i owns batch i//4 and
heads 3*(i%4)..3*(i%4)+2.  Projections + attention are fully local (weights
row-sliced on the host).  A per-head 8-way AllToAll redistributes the
context so core i owns query rows 256*i..256*(i+1) of BOTH batches, and the
output projection accumulates PER A2A ROUND (4 global heads at a time, Wo
row-permuted dh-major on the host), leaving only the last head's A2A on the
critical path -- and that one is split into two d-half pieces pipelined
against its own output projection.

Schedule: the attention inner loop is ACT(exp)-bound (~1.15us per key-block
covering 2 q-chunks, [128,1024] exp tiles on a 2-deep PSUM ring).  All
other PE work (later heads' Q/K projections, per-round output projection,
reciprocal broadcasts) is interleaved one small "extra" per iteration so
the PE stream stays dense without out-running the ACT.  Softmax
normalization computes 1/d = exp(-ln(d)) on the ACT (ln+exp+identity share
one activation table set, pinned at compile time so walrus emits a single
ACT_TABLE_LOAD) and broadcasts the reciprocal row across partitions with a
tiny [2,64]-selector matmul -- no DRAM round-trip, no slow DVE reciprocal.
"""

import numpy as np
import ml_dtypes

import concourse.mybir as mybir
import concourse.tile as tile
from concourse import bacc
from concourse.bass_utils import run_bass_kernel_spmd

B, N, C, H, HD = 2, 2048, 768, 12, 64
SCALE = HD ** -0.5          # 0.125
P = 128
CB = C // P                 # 6 contraction blocks of 128 over channels
KB = N // P                 # 16 key blocks
QCH = 512                   # query chunk (max moving free dim)
HPC = 3                     # heads per core
NCORES = 8
VW = HPC * (HD + 1)         # 195: v columns per key-block (3 heads + ones col)
RQ = N // NCORES            # 256 query rows per core per batch after A2A
PVLAG = 6                   # PV trails scores by this many key-blocks
# iterations from A2A staging to its out-proj work (waits for the
# collective to land so the PE stream never head-of-line blocks on it)
OUTPROJ_DELAY = {0: 38, 1: 28}

f32 = mybir.dt.float32
bf16 = mybir.dt.bfloat16
Exp = mybir.ActivationFunctionType.Exp
Log = mybir.ActivationFunctionType.Ln
Identity = mybir.ActivationFunctionType.Identity

# head -> (block, partition offset) inside qT_sb / kT_sb [128, 2*2048].
HOFF = {0: (0, 0), 1: (0, 64), 2: (1, 0)}
# wqkT host column order: [q0 q1 | k0 k1 | q2 | k2]
# (col offset, m, dest 'q' or 'k', dest block)
PASS_K01 = (128, 128, "k", 0)
PASS_Q01 = (0, 128, "q", 0)
PASS_Q2 = (256, 64, "q", 1)
PASS_K2 = (320, 64, "k", 1)


def _body(nc, tc, xT, wqkT, wvT, woTp, bo_d, out_d):
    with (
        tc.tile_pool(name="const", bufs=1) as constp,
        tc.tile_pool(name="big", bufs=1) as bigp,
        tc.tile_pool(name="esp", bufs=PVLAG + 10) as esp,
        tc.tile_pool(name="smallp", bufs=4) as smallp,
        tc.tile_pool(name="ctxfp", bufs=2) as ctxfp,
        tc.tile_pool(name="outp", bufs=2) as outp,
        tc.tile_pool(name="psS", bufs=2, space="PSUM") as psS,
        tc.tile_pool(name="psC", bufs=3, space="PSUM") as psC,
        tc.tile_pool(name="psM", bufs=1, space="PSUM") as psM,
        tc.tile_pool(name="dram", bufs=1, space="DRAM") as dramp,
    ):
        # PSUM budget (8 banks): psS 2x[128,1024] (4) scores + lead-in
        # projections; psC 3x[65,512] (3) PV accumulators (2 live per
        # q-half) + reciprocal-broadcast tiles; psM 1x[128,512] (1) for the
        # interleaved projection / output-projection groups.  All psM/psC
        # tiles are allocated in CONSUMPTION order (lazily where needed) so
        # the pool ring dependencies match the emission order.

        # ---- load inputs to SBUF (all bf16 except bias) ----
        xT_sb = [bigp.tile([P, N], bf16, name=f"xT_sb_{cb}") for cb in range(CB)]
        wqkT_sb = bigp.tile([P, CB * 384], bf16, name="wqkT_sb")
        wvT_sb = bigp.tile([P, CB * 192], bf16, name="wvT_sb")
        woTp_sb = bigp.tile([P, CB * C], bf16, name="woTp_sb")
        bo_sb = bigp.tile([P, CB], f32, name="bo_sb")
        ones_sb = constp.tile([P, 64], bf16, name="ones_sb")
        nc.vector.memset(ones_sb[:, :], 1.0)
        # selector for the reciprocal broadcast matmul: cols [qc*64, qc*64+64)
        # form a [2,64] tile whose row qc is ones.  Engines cannot address
        # partition 1 alone, so row 1 is filled via sbuf->sbuf DMA.
        selsrc = constp.tile([1, 128], bf16, name="selsrc")
        nc.vector.memset(selsrc[0:1, 0:64], 1.0)
        nc.vector.memset(selsrc[0:1, 64:128], 0.0)
        sel_sb = constp.tile([2, 128], bf16, name="sel_sb")
        nc.sync.dma_start(sel_sb[0:1, :], selsrc[0:1, :])
        nc.sync.dma_start(sel_sb[1:2, 0:64], selsrc[0:1, 64:128])
        nc.sync.dma_start(sel_sb[1:2, 64:128], selsrc[0:1, 0:64])
        zero_sb = constp.tile([P, QCH], f32, name="zero_sb")
        nc.vector.memset(zero_sb[:, :], 0.0)
        # warm the ACT ln+exp table set (one-time PSEUDO_LOAD) during loads
        warm_sb = constp.tile([P, 2], f32, name="warm_sb")
        nc.scalar.activation(warm_sb[0:1, 0:1], ones_sb[0:1, 0:1], Exp, scale=SCALE)
        nc.scalar.activation(warm_sb[0:1, 1:2], ones_sb[0:1, 0:1], Log)

        # loads split over three DGE queues; x gates the whole lead-in so
        # it is spread over all three.
        for cb in range(CB):
            nc.scalar.dma_start(wqkT_sb[:, cb * 384:(cb + 1) * 384], wqkT[cb * P:(cb + 1) * P, :])
        for cb in range(CB):
            nc.scalar.dma_start(bo_sb[:, cb:cb + 1], bo_d[cb * P:(cb + 1) * P, :])
        for cb in range(CB):
            eng = nc.sync if cb % 2 == 0 else nc.gpsimd
            eng.dma_start(xT_sb[cb][:, :], xT[cb * P:(cb + 1) * P, :])
            nc.scalar.dma_start(wvT_sb[:, cb * 192:(cb + 1) * 192], wvT[cb * P:(cb + 1) * P, :])
        for cb in range(CB):
            nc.scalar.dma_start(woTp_sb[:, cb * C:(cb + 1) * C], woTp[cb * P:(cb + 1) * P, :])

        # bias broadcast [128, 6*512] f32 built once while ACT is idle; it
        # seeds the output-projection accumulator chain.
        bias_bc = bigp.tile([P, CB * QCH], f32, name="bias_bc")
        for cbo in range(CB):
            nc.scalar.activation(bias_bc[:, cbo * QCH:(cbo + 1) * QCH],
                                 zero_sb[:, :], Identity, bias=bo_sb[:, cbo:cbo + 1])

        qT_sb = bigp.tile([P, 2 * N], bf16, name="qT_sb")
        kT_sb = bigp.tile([P, 2 * N], bf16, name="kT_sb")
        v_sb = bigp.tile([P, KB * VW], bf16, name="v_sb")
        ctxT_sb = bigp.tile([64, HPC * N], bf16, name="ctxT_sb")
        accA = bigp.tile([P, CB * QCH], f32, name="accA")
        accB = bigp.tile([P, CB * QCH], f32, name="accB")
        dtmp_sb = bigp.tile([65, 1024], f32, name="dtmp_sb")

        # ---- emission helpers ------------------------------------------
        def qk_group_psS(co, m, dst, blk, qp):
            """Lead-in only: one [m, 1024] projection group on the psS ring
            (12 matmuls + DVE drain), emitted immediately."""
            dst_sb = qT_sb if dst == "q" else kT_sb
            ps = psS.tile([P, 2 * QCH], f32, name=f"pj_{dst}_{blk}_{qp}", tag="psS")
            for half in range(2):
                qn = qp * 2 + half
                for cb in range(CB):
                    nc.tensor.matmul(
                        ps[:m, half * QCH:(half + 1) * QCH],
                        lhsT=wqkT_sb[:, cb * 384 + co: cb * 384 + co + m],
                        rhs=xT_sb[cb][:, qn * QCH: qn * QCH + QCH],
                        start=(cb == 0), stop=(cb == CB - 1),
                    )
            nc.vector.tensor_copy(
                dst_sb[:m, blk * N + qp * 2 * QCH: blk * N + (qp + 1) * 2 * QCH],
                ps[:m, :])

        def qk_group512(co, m, dst, blk, qn):
            """Deferred [m, 512] projection group on psM: 3 closures."""
            dst_sb = qT_sb if dst == "q" else kT_sb
            cell = []

            def mm0():
                cell.append(psM.tile([P, QCH], f32, name=f"pj2_{dst}_{blk}_{qn}", tag="psM"))
                for cb in range(3):
                    nc.tensor.matmul(
                        cell[0][:m, 0:QCH],
                        lhsT=wqkT_sb[:, cb * 384 + co: cb * 384 + co + m],
                        rhs=xT_sb[cb][:, qn * QCH: qn * QCH + QCH],
                        start=(cb == 0), stop=False,
                    )

            def mm1():
                for cb in range(3, CB):
                    nc.tensor.matmul(
                        cell[0][:m, 0:QCH],
                        lhsT=wqkT_sb[:, cb * 384 + co: cb * 384 + co + m],
                        rhs=xT_sb[cb][:, qn * QCH: qn * QCH + QCH],
                        start=False, stop=(cb == CB - 1),
                    )

            def drain():
                nc.vector.tensor_copy(
                    dst_sb[:m, blk * N + qn * QCH: blk * N + (qn + 1) * QCH],
                    cell[0][:m, 0:QCH])
            return [mm0, mm1, drain]

        def v_group(nb):
            ps = psM.tile([P, QCH], f32, name=f"vps_{nb}", tag="psM")
            for cb in range(CB):
                nc.tensor.matmul(
                    ps[:, 0:192],
                    lhsT=xT_sb[cb][:, nb * P:(nb + 1) * P],
                    rhs=wvT_sb[:, cb * 192:(cb + 1) * 192],
                    start=(cb == 0), stop=(cb == CB - 1),
                )
            vv = v_sb[:, nb * VW:(nb + 1) * VW].rearrange("p (h w) -> p h w", h=HPC)
            pp = ps[:, 0:192].rearrange("p (h w) -> p h w", h=HPC)
            nc.vector.tensor_copy(vv[:, :, 0:64], pp[:, :, :])
            nc.vector.memset(vv[:, :, 64:65], 1.0)

        # ---- lead-in: k0/k1 (all kb), q0/q1 chunks 0-1, all of V --------
        co, m, dst, blk = PASS_K01
        for qp in range(2):
            qk_group_psS(co, m, dst, blk, qp)
        co, m, dst, blk = PASS_Q01
        qk_group_psS(co, m, dst, blk, 0)
        for nb in range(KB):
            v_group(nb)

        # ---- deferred extras (consumed one per attention iteration);
        # ordered by deadline: q0/q1 chunks 2-3 by iter 16, q2/k2 by 64.
        extras = []
        co, m, dst, blk = PASS_Q01
        for qn in (2, 3):
            extras.extend(qk_group512(co, m, dst, blk, qn))
        for co, m, dst, blk in (PASS_Q2, PASS_K2):
            for qn in range(4):
                extras.extend(qk_group512(co, m, dst, blk, qn))

        front = []

        def consume():
            if front:
                front.pop(0)()
            elif extras:
                extras.pop(0)()

        # ---- attention helpers -----------------------------------------
        def make_pv(h, cps, kb, es):
            def run():
                for qx in range(2):
                    nc.tensor.matmul(
                        cps[qx][0:65, :],
                        lhsT=v_sb[:, kb * VW + h * 65: kb * VW + (h + 1) * 65],
                        rhs=es[:, qx * QCH:(qx + 1) * QCH],
                        start=(kb == 0), stop=(kb == KB - 1),
                    )
            return run

        def norm_closures(h, f, cps):
            """Normalize the finished half into ctxT_sb.  bps tiles are
            allocated eagerly so the psC ring order (cps0, cps1, bps0, bps1
            per half) matches consumption order."""
            rec_in = smallp.tile([2, QCH], f32, name=f"rin_{h}_{f}", tag="rin")
            lt = smallp.tile([2, QCH], f32, name=f"lt_{h}_{f}", tag="lt")
            rec = smallp.tile([2, QCH], bf16, name=f"rec_{h}_{f}", tag="rec")
            rb = [smallp.tile([64, QCH], bf16, name=f"rb{qx}_{h}_{f}", tag=f"rb{qx}")
                  for qx in range(2)]
            bps0 = psC.tile([65, QCH], f32, name=f"bps0_{h}_{f}", tag="psC")
            bps1 = psC.tile([65, QCH], f32, name=f"bps1_{h}_{f}", tag="psC")

            def denoms():
                for qx in range(2):
                    nc.vector.tensor_copy(
                        dtmp_sb[64:65, qx * QCH:(qx + 1) * QCH],
                        cps[qx][64:65, :])
                nc.sync.dma_start(rec_in[0:1, :], dtmp_sb[64:65, 0:QCH])
                nc.sync.dma_start(rec_in[1:2, :], dtmp_sb[64:65, QCH:2 * QCH])

            def lnexp():
                nc.scalar.activation(lt, rec_in, Log)
                nc.scalar.activation(rec, lt, Exp, scale=-1.0)
                nc.tensor.matmul(bps0[0:64, :], lhsT=sel_sb[:, 0:64], rhs=rec[:, :],
                                 start=True, stop=True)

            def mul0():
                qc = 2 * f
                nc.vector.tensor_copy(rb[0][:, :], bps0[0:64, :])
                nc.tensor.matmul(bps1[0:64, :], lhsT=sel_sb[:, 64:128], rhs=rec[:, :],
                                 start=True, stop=True)
                nc.vector.tensor_mul(
                    ctxT_sb[0:64, h * N + qc * QCH: h * N + (qc + 1) * QCH],
                    cps[0][0:64, :], rb[0][:, :])

            def mul1():
                qc = 2 * f + 1
                nc.vector.tensor_copy(rb[1][:, :], bps1[0:64, :])
                nc.vector.tensor_mul(
                    ctxT_sb[0:64, h * N + qc * QCH: h * N + (qc + 1) * QCH],
                    cps[1][0:64, :], rb[1][:, :])
                # the previous half's psC slots are now fully released:
                # PV pops for the new half may resume (see boundary_ok)
                boundary_ok[0] = True
            return [denoms, lnexp, mul0, mul1]

        def a2a_stage(h):
            # ctxf row = s4*32 + dl, block dh at cols dh*512 (dh-major so
            # the last head's A2A can be split into two d-row pieces).
            # Recvs ride the gpsimd queue -- on sync they head-of-line
            # block the next boundary's DMAs behind the collective sem.
            send_h = dramp.tile([NCORES, 64, RQ], bf16, name=f"send_{h}")
            recv_h = dramp.tile([NCORES, 64, RQ], bf16, name=f"recv_{h}")
            ctxf = ctxfp.tile([P, 2 * QCH], bf16, name=f"ctxf_{h}", tag="ctxf")

            def stage():
                for j in range(NCORES):
                    nc.sync.dma_start(send_h[j, :, :],
                                      ctxT_sb[:, h * N + j * RQ: h * N + (j + 1) * RQ])
                nc.gpsimd.collective_compute(
                    "AllToAll", mybir.AluOpType.bypass,
                    replica_groups=[list(range(NCORES))],
                    ins=[send_h.opt()], outs=[recv_h.opt()],
                )
                for s in range(NCORES):
                    s4 = s % 4
                    for dh in range(2):
                        nc.gpsimd.dma_start(
                            ctxf[s4 * 32: s4 * 32 + 32,
                                 dh * QCH + (s // 4) * RQ: dh * QCH + (s // 4) * RQ + RQ],
                            recv_h[s, dh * 32:(dh + 1) * 32, :])
            return stage, ctxf

        def outproj_closures(r, ctxf, src_acc, dst_acc):
            cls = []
            for cbo in range(CB):
                def run(cbo=cbo):
                    ps = psM.tile([P, QCH], f32, name=f"ops_{r}_{cbo}", tag="psM")
                    for jb in range(2):
                        nc.tensor.matmul(
                            ps[:, :],
                            lhsT=woTp_sb[:, (2 * r + jb) * C + cbo * P: (2 * r + jb) * C + (cbo + 1) * P],
                            rhs=ctxf[:, jb * QCH:(jb + 1) * QCH],
                            start=(jb == 0), stop=(jb == 1),
                        )
                    nc.vector.tensor_add(
                        dst_acc[:, cbo * QCH:(cbo + 1) * QCH],
                        ps[:, :], src_acc[:, cbo * QCH:(cbo + 1) * QCH])
                cls.append(run)
            return cls

        # ---- main attention loop ---------------------------------------
        acc_chain = [(bias_bc, accB), (accB, accA)]
        pending_a2a = []  # (round, ctxf, release_iter)
        iter_no = 0
        last_cps = None
        # PV matmuls for a new half may only be emitted once the previous
        # half's normalization (which releases the psC ring slots via PE
        # broadcast matmuls emitted in the norm closures) is in the stream;
        # otherwise the in-order PE queue deadlocks on the slot wait.
        boundary_ok = [True]

        for h in range(HPC):
            hb_, ho_ = HOFF[h]
            for f in range(2):
                cps = [psC.tile([65, QCH], f32, name=f"cps_{h}_{f}_{qx}", tag="psC")
                       for qx in range(2)]
                pvq = []
                for kb in range(KB):
                    sps = psS.tile([P, 2 * QCH], f32, name=f"sps_{h}_{f}_{kb}", tag="psS")
                    for qx in range(2):
                        qc = 2 * f + qx
                        nc.tensor.matmul(
                            sps[:, qx * QCH:(qx + 1) * QCH],
                            lhsT=kT_sb[ho_:ho_ + 64, hb_ * N + kb * P: hb_ * N + (kb + 1) * P],
                            rhs=qT_sb[ho_:ho_ + 64, hb_ * N + qc * QCH: hb_ * N + qc * QCH + QCH],
                            start=True, stop=True,
                        )
                    es = esp.tile([P, 2 * QCH], bf16, name=f"es_{h}_{f}_{kb}", tag="es")
                    nc.scalar.activation(es, sps, Exp, scale=SCALE)
                    pvq.append(make_pv(h, cps, kb, es))
                    if pending_a2a and iter_no >= pending_a2a[0][2]:
                        r, ctxf, _ = pending_a2a.pop(0)
                        src, dst = acc_chain[r]
                        front.extend(outproj_closures(r, ctxf, src, dst))
                    consume()
                    if len(pvq) > PVLAG and boundary_ok[0]:
                        pvq.pop(0)()
                    iter_no += 1
                # end of half: flush the pending PVs two per slot, then
                # normalization (+ A2A staging when the head is done).
                for i in range(0, len(pvq), 2):
                    pair = pvq[i:i + 2]
                    front.append(lambda pair=pair: [c() for c in pair])
                pvq = []
                if h == HPC - 1 and f == 1:
                    last_cps = cps  # tail handles the final norm + A2A
                else:
                    boundary_ok[0] = False
                    front.extend(norm_closures(h, f, cps))
                    if f == 1:
                        stage, ctxf = a2a_stage(h)
                        front.append(stage)
                        pending_a2a.append((h, ctxf, iter_no + OUTPROJ_DELAY[h]))

        # ---- tail: final norm split by d-half, A2A in two pipelined
        # pieces, sends/recvs/stores spread over the idle queues ----------
        while front or extras:
            consume()
        r = HPC - 1
        cps = last_cps
        rec_in = smallp.tile([2, QCH], f32, name="rin_T", tag="rin")
        lt = smallp.tile([2, QCH], f32, name="lt_T", tag="lt")
        rec = smallp.tile([2, QCH], bf16, name="rec_T", tag="rec")
        rb = [smallp.tile([64, QCH], bf16, name=f"rb{qx}_T", tag=f"rb{qx}")
              for qx in range(2)]
        bps0 = psC.tile([65, QCH], f32, name="bps0_T", tag="psC")
        bps1 = psC.tile([65, QCH], f32, name="bps1_T", tag="psC")
        for qx in range(2):
            nc.vector.tensor_copy(dtmp_sb[64:65, qx * QCH:(qx + 1) * QCH],
                                  cps[qx][64:65, :])
        nc.sync.dma_start(rec_in[0:1, :], dtmp_sb[64:65, 0:QCH])
        nc.sync.dma_start(rec_in[1:2, :], dtmp_sb[64:65, QCH:2 * QCH])
        nc.scalar.activation(lt, rec_in, Log)
        nc.scalar.activation(rec, lt, Exp, scale=-1.0)
        nc.tensor.matmul(bps0[0:64, :], lhsT=sel_sb[:, 0:64], rhs=rec[:, :],
                         start=True, stop=True)
        nc.vector.tensor_copy(rb[0][:, :], bps0[0:64, :])
        nc.tensor.matmul(bps1[0:64, :], lhsT=sel_sb[:, 64:128], rhs=rec[:, :],
                         start=True, stop=True)
        nc.vector.tensor_copy(rb[1][:, :], bps1[0:64, :])

        ctxf_t = ctxfp.tile([P, 2 * QCH], bf16, name="ctxf_T", tag="ctxf")
        send_p = [dramp.tile([NCORES, 32, RQ], bf16, name=f"sendT_{dh}") for dh in range(2)]
        recv_p = [dramp.tile([NCORES, 32, RQ], bf16, name=f"recvT_{dh}") for dh in range(2)]
        for dh in range(2):
            # normalize only this d-half's 32 rows, stage, and trigger --
            # the lo piece's collective starts while the hi rows multiply
            for qx in range(2):
                qc = 2 + qx
                nc.vector.tensor_mul(
                    ctxT_sb[dh * 32:(dh + 1) * 32, r * N + qc * QCH: r * N + (qc + 1) * QCH],
                    cps[qx][dh * 32:(dh + 1) * 32, :], rb[qx][dh * 32:(dh + 1) * 32, :])
            for j in range(NCORES):
                eng = nc.sync if j % 2 == 0 else nc.scalar
                eng.dma_start(send_p[dh][j, :, :],
                              ctxT_sb[dh * 32:(dh + 1) * 32, r * N + j * RQ: r * N + (j + 1) * RQ])
            nc.gpsimd.collective_compute(
                "AllToAll", mybir.AluOpType.bypass,
                replica_groups=[list(range(NCORES))],
                ins=[send_p[dh].opt()], outs=[recv_p[dh].opt()],
            )
        for dh in range(2):
            engs = [nc.gpsimd, nc.scalar, nc.sync]
            for s in range(NCORES):
                s4 = s % 4
                engs[s % 3].dma_start(
                    ctxf_t[s4 * 32: s4 * 32 + 32,
                           dh * QCH + (s // 4) * RQ: dh * QCH + (s // 4) * RQ + RQ],
                    recv_p[dh][s, :, :])
            src = accA if dh == 0 else accB
            for cbo in range(CB):
                ps = psS.tile([P, 2 * QCH], f32, name=f"opsT_{dh}_{cbo}", tag="psS")
                nc.tensor.matmul(
                    ps[:, 0:QCH],
                    lhsT=woTp_sb[:, (2 * r + dh) * C + cbo * P: (2 * r + dh) * C + (cbo + 1) * P],
                    rhs=ctxf_t[:, dh * QCH:(dh + 1) * QCH],
                    start=True, stop=True,
                )
                if dh == 0:
                    nc.vector.tensor_add(
                        accB[:, cbo * QCH:(cbo + 1) * QCH],
                        ps[:, 0:QCH], src[:, cbo * QCH:(cbo + 1) * QCH])
                else:
                    osb = outp.tile([P, QCH], f32, name=f"osb_{cbo}", tag="osb")
                    nc.vector.tensor_add(osb, ps[:, 0:QCH], src[:, cbo * QCH:(cbo + 1) * QCH])
                    eng = nc.sync if cbo % 2 == 0 else nc.scalar
                    eng.dma_start(out_d[cbo * P:(cbo + 1) * P, :], osb)


_JOINT_SET = "natural_log_exp_and_others"


def _pin_act_tables():
    """Restrict the activation-table chooser to the one set containing
    exp+ln+identity, so the compiled stream has a single ACT_TABLE_LOAD
    instead of flip-flopping between exp_and_others and natural_log (a
    ~1.3us stall at every half boundary).  Only the table *selection* for
    this kernel's own compile is affected; list positions (the
    act_func_set_id encoding) are preserved."""
    orig = bacc.get_activation_tables

    def pinned(arch):
        t = orig(arch)
        if _JOINT_SET not in t:
            return t
        return {k: (v if k == _JOINT_SET else set()) for k, v in t.items()}

    bacc.get_activation_tables = pinned
    return orig


def build():
    orig_tables = _pin_act_tables()
    try:
        return _build()
    finally:
        bacc.get_activation_tables = orig_tables


def _build():
    nc = bacc.Bacc("TRN2", target_bir_lowering=False, debug=False, num_devices=NCORES)
    xT = nc.dram_tensor("xT", [C, N], bf16, kind="ExternalInput").ap()
    wqkT = nc.dram_tensor("wqkT", [C, 2 * HPC * HD], bf16, kind="ExternalInput").ap()
    wvT = nc.dram_tensor("wvT", [C, HPC * HD], bf16, kind="ExternalInput").ap()
    woTp = nc.dram_tensor("woTp", [C, C], bf16, kind="ExternalInput").ap()
    bo_d = nc.dram_tensor("bo", [C, 1], f32, kind="ExternalInput").ap()
    out_d = nc.dram_tensor("out", [C, 2 * RQ], f32, kind="ExternalOutput").ap()
    with tile.TileContext(nc) as tc:
        _body(nc, tc, xT, wqkT, wvT, woTp, bo_d, out_d)
    nc.compile()
    return nc


_NC = None


def _get_nc():
    global _NC
    if _NC is None:
        _NC = build()
    return _NC


# Wo row permutation (dh-major): A2A round r, d-half dh delivers global
# heads {s4*3+r} rows [dh*32,(dh+1)*32) as contiguous channel block
# r*256 + dh*128 + s4*32 + dl.
_PERM = np.array([(s4 * 3 + r) * 64 + dh * 32 + dl
                  for r in range(HPC) for dh in range(2)
                  for s4 in range(4) for dl in range(32)])


def make_in_maps(x, Wq, Wk, Wv, Wo, bo):
    x = np.asarray(x, np.float32)
    woTp = np.ascontiguousarray(np.asarray(Wo, np.float32).T[_PERM, :]).astype(ml_dtypes.bfloat16)
    bo_col = np.ascontiguousarray(np.asarray(bo, np.float32).reshape(C, 1))
    in_maps = []
    for i in range(NCORES):
        b = i // 4
        hs = (i % 4) * HPC
        rq = slice(hs * HD, (hs + HPC) * HD)
        wq_s = np.asarray(Wq, np.float32)[rq]  # [192, 768]
        wk_s = np.asarray(Wk, np.float32)[rq]
        # column order: [q0 q1 | k0 k1 | q2 | k2]
        wqk = np.concatenate([wq_s[0:128], wk_s[0:128], wq_s[128:192], wk_s[128:192]], axis=0).T
        in_maps.append({
            "xT": np.ascontiguousarray(x[b].T).astype(ml_dtypes.bfloat16),
            "wqkT": np.ascontiguousarray(wqk).astype(ml_dtypes.bfloat16),
            "wvT": np.ascontiguousarray(np.asarray(Wv, np.float32)[rq].T).astype(ml_dtypes.bfloat16),
            "woTp": woTp,
            "bo": bo_col,
        })
    return in_maps


def unshard(results):
    out = np.empty((B, N, C), np.float32)
    for i, r in enumerate(results):
        o = r["out"]  # [768, 512]: cols 0-255 batch 0, 256-511 batch 1
        out[0, i * RQ:(i + 1) * RQ, :] = o[:, :RQ].T
        out[1, i * RQ:(i + 1) * RQ, :] = o[:, RQ:].T
    return out


def kernel(x, Wq, Wk, Wv, Wo, bo):
    nc = _get_nc()
    in_maps = make_in_maps(x, Wq, Wk, Wv, Wo, bo)
    res = run_bass_kernel_spmd(nc, in_maps, core_ids=list(range(NCORES)))
    return unshard(res.results)


# revision 38
# speedup vs baseline: 1.0845x; 1.0078x over previous
"""Distributed multi-head attention kernel for 8 TRN2 NeuronCores.

Problem: B=2, N=2048, C=768, H=12 heads of dim 64.
  q = x @ Wq.T ; k = x @ Wk.T ; v = x @ Wv.T      (per-head split)
  out = softmax(q k^T / 8) v                        (full N^2 attention)
  y = concat_heads(out) @ Wo.T + bo

Sharding: 24 (batch, head) pairs -> 3 per core.  Core i owns batch i//4 and
heads 3*(i%4)..3*(i%4)+2.  Projections + attention are fully local (weights
row-sliced on the host).  A per-head 8-way AllToAll redistributes the
context so core i owns query rows 256*i..256*(i+1) of BOTH batches, and the
output projection accumulates PER A2A ROUND (4 global heads at a time, Wo
row-permuted dh-major on the host), leaving only the last head's A2A on the
critical path -- and that one is split into two d-half pieces pipelined
against its own output projection.

Schedule: the attention inner loop is ACT(exp)-bound (~1.15us per key-block
covering 2 q-chunks, [128,1024] exp tiles on a 2-deep PSUM ring).  All
other PE work (later heads' Q/K projections, per-round output projection,
reciprocal broadcasts) is interleaved one small "extra" per iteration so
the PE stream stays dense without out-running the ACT.  Softmax
normalization computes 1/d = exp(-ln(d)) on the ACT (ln+exp+identity share
one activation table set, pinned at compile time so walrus emits a single
ACT_TABLE_LOAD) and broadcasts the reciprocal row across partitions with a
tiny [2,64]-selector matmul -- no DRAM round-trip, no slow DVE reciprocal.
"""

import numpy as np
import ml_dtypes

import concourse.mybir as mybir
import concourse.tile as tile
from concourse import bacc
from concourse.bass_utils import run_bass_kernel_spmd

B, N, C, H, HD = 2, 2048, 768, 12, 64
SCALE = HD ** -0.5          # 0.125
P = 128
CB = C // P                 # 6 contraction blocks of 128 over channels
KB = N // P                 # 16 key blocks
QCH = 512                   # query chunk (max moving free dim)
HPC = 3                     # heads per core
NCORES = 8
VW = HPC * (HD + 1)         # 195: v columns per key-block (3 heads + ones col)
RQ = N // NCORES            # 256 query rows per core per batch after A2A
PVLAG = 6                   # PV trails scores by this many key-blocks
# iterations from A2A staging to its out-proj work (waits for the
# collective to land so the PE stream never head-of-line blocks on it)
OUTPROJ_DELAY = {0: 38, 1: 28}

f32 = mybir.dt.float32
bf16 = mybir.dt.bfloat16
Exp = mybir.ActivationFunctionType.Exp
Log = mybir.ActivationFunctionType.Ln
Identity = mybir.ActivationFunctionType.Identity

# head -> (block, partition offset) inside qT_sb / kT_sb [128, 2*2048].
HOFF = {0: (0, 0), 1: (0, 64), 2: (1, 0)}
# wqkT host column order: [q0 q1 | k0 k1 | q2 | k2]
# (col offset, m, dest 'q' or 'k', dest block)
PASS_K01 = (128, 128, "k", 0)
PASS_Q01 = (0, 128, "q", 0)
PASS_Q2 = (256, 64, "q", 1)
PASS_K2 = (320, 64, "k", 1)


def _body(nc, tc, xT, wqkT, wvT, woTp, bo_d, out_d):
    with (
        tc.tile_pool(name="const", bufs=1) as constp,
        tc.tile_pool(name="big", bufs=1) as bigp,
        tc.tile_pool(name="esp", bufs=PVLAG + 10) as esp,
        tc.tile_pool(name="smallp", bufs=4) as smallp,
        tc.tile_pool(name="ctxfp", bufs=2) as ctxfp,
        tc.tile_pool(name="outp", bufs=2) as outp,
        tc.tile_pool(name="psS", bufs=2, space="PSUM") as psS,
        tc.tile_pool(name="psC", bufs=3, space="PSUM") as psC,
        tc.tile_pool(name="psM", bufs=1, space="PSUM") as psM,
        tc.tile_pool(name="dram", bufs=1, space="DRAM") as dramp,
    ):
        # PSUM budget (8 banks): psS 2x[128,1024] (4) scores + lead-in
        # projections; psC 3x[65,512] (3) PV accumulators (2 live per
        # q-half) + reciprocal-broadcast tiles; psM 1x[128,512] (1) for the
        # interleaved projection / output-projection groups.  All psM/psC
        # tiles are allocated in CONSUMPTION order (lazily where needed) so
        # the pool ring dependencies match the emission order.

        # ---- load inputs to SBUF (all bf16 except bias) ----
        xT_sb = [bigp.tile([P, N], bf16, name=f"xT_sb_{cb}") for cb in range(CB)]
        wqkT_sb = bigp.tile([P, CB * 384], bf16, name="wqkT_sb")
        wvT_sb = bigp.tile([P, CB * 192], bf16, name="wvT_sb")
        woTp_sb = bigp.tile([P, CB * C], bf16, name="woTp_sb")
        bo_sb = bigp.tile([P, CB], f32, name="bo_sb")
        ones_sb = constp.tile([P, 64], bf16, name="ones_sb")
        nc.vector.memset(ones_sb[:, :], 1.0)
        # selector for the reciprocal broadcast matmul: cols [qc*64, qc*64+64)
        # form a [2,64] tile whose row qc is ones.  Engines cannot address
        # partition 1 alone, so row 1 is filled via sbuf->sbuf DMA.
        selsrc = constp.tile([1, 128], bf16, name="selsrc")
        nc.vector.memset(selsrc[0:1, 0:64], 1.0)
        nc.vector.memset(selsrc[0:1, 64:128], 0.0)
        sel_sb = constp.tile([2, 128], bf16, name="sel_sb")
        nc.sync.dma_start(sel_sb[0:1, :], selsrc[0:1, :])
        nc.sync.dma_start(sel_sb[1:2, 0:64], selsrc[0:1, 64:128])
        nc.sync.dma_start(sel_sb[1:2, 64:128], selsrc[0:1, 0:64])
        zero_sb = constp.tile([P, QCH], f32, name="zero_sb")
        nc.vector.memset(zero_sb[:, :], 0.0)
        # warm the ACT ln+exp table set (one-time PSEUDO_LOAD) during loads
        warm_sb = constp.tile([P, 2], f32, name="warm_sb")
        nc.scalar.activation(warm_sb[0:1, 0:1], ones_sb[0:1, 0:1], Exp, scale=SCALE)
        nc.scalar.activation(warm_sb[0:1, 1:2], ones_sb[0:1, 0:1], Log)

        # loads split over three DGE queues; x gates the whole lead-in so
        # it is spread over all three.
        for cb in range(CB):
            nc.scalar.dma_start(wqkT_sb[:, cb * 384:(cb + 1) * 384], wqkT[cb * P:(cb + 1) * P, :])
        for cb in range(CB):
            nc.scalar.dma_start(bo_sb[:, cb:cb + 1], bo_d[cb * P:(cb + 1) * P, :])
        for cb in range(CB):
            eng = nc.sync if cb % 2 == 0 else nc.gpsimd
            eng.dma_start(xT_sb[cb][:, :], xT[cb * P:(cb + 1) * P, :])
            nc.scalar.dma_start(wvT_sb[:, cb * 192:(cb + 1) * 192], wvT[cb * P:(cb + 1) * P, :])
        for cb in range(CB):
            nc.scalar.dma_start(woTp_sb[:, cb * C:(cb + 1) * C], woTp[cb * P:(cb + 1) * P, :])

        # bias broadcast [128, 6*512] f32 built once while ACT is idle; it
        # seeds the output-projection accumulator chain.
        bias_bc = bigp.tile([P, CB * QCH], f32, name="bias_bc")
        for cbo in range(CB):
            nc.scalar.activation(bias_bc[:, cbo * QCH:(cbo + 1) * QCH],
                                 zero_sb[:, :], Identity, bias=bo_sb[:, cbo:cbo + 1])

        qT_sb = bigp.tile([P, 2 * N], bf16, name="qT_sb")
        kT_sb = bigp.tile([P, 2 * N], bf16, name="kT_sb")
        v_sb = bigp.tile([P, KB * VW], bf16, name="v_sb")
        ctxT_sb = bigp.tile([64, HPC * N], bf16, name="ctxT_sb")
        accA = bigp.tile([P, CB * QCH], f32, name="accA")
        accB = bigp.tile([P, CB * QCH], f32, name="accB")
        dtmp_sb = bigp.tile([65, 1024], f32, name="dtmp_sb")

        # ---- emission helpers ------------------------------------------
        def qk_group_psS(co, m, dst, blk, qp):
            """Lead-in only: one [m, 1024] projection group on the psS ring
            (12 matmuls + DVE drain), emitted immediately."""
            dst_sb = qT_sb if dst == "q" else kT_sb
            ps = psS.tile([P, 2 * QCH], f32, name=f"pj_{dst}_{blk}_{qp}", tag="psS")
            for half in range(2):
                qn = qp * 2 + half
                for cb in range(CB):
                    nc.tensor.matmul(
                        ps[:m, half * QCH:(half + 1) * QCH],
                        lhsT=wqkT_sb[:, cb * 384 + co: cb * 384 + co + m],
                        rhs=xT_sb[cb][:, qn * QCH: qn * QCH + QCH],
                        start=(cb == 0), stop=(cb == CB - 1),
                    )
            nc.vector.tensor_copy(
                dst_sb[:m, blk * N + qp * 2 * QCH: blk * N + (qp + 1) * 2 * QCH],
                ps[:m, :])

        def qk_group512(co, m, dst, blk, qn):
            """Deferred [m, 512] projection group on psM: 3 closures."""
            dst_sb = qT_sb if dst == "q" else kT_sb
            cell = []

            def mm0():
                cell.append(psM.tile([P, QCH], f32, name=f"pj2_{dst}_{blk}_{qn}", tag="psM"))
                for cb in range(3):
                    nc.tensor.matmul(
                        cell[0][:m, 0:QCH],
                        lhsT=wqkT_sb[:, cb * 384 + co: cb * 384 + co + m],
                        rhs=xT_sb[cb][:, qn * QCH: qn * QCH + QCH],
                        start=(cb == 0), stop=False,
                    )

            def mm1():
                for cb in range(3, CB):
                    nc.tensor.matmul(
                        cell[0][:m, 0:QCH],
                        lhsT=wqkT_sb[:, cb * 384 + co: cb * 384 + co + m],
                        rhs=xT_sb[cb][:, qn * QCH: qn * QCH + QCH],
                        start=False, stop=(cb == CB - 1),
                    )

            def drain():
                nc.vector.tensor_copy(
                    dst_sb[:m, blk * N + qn * QCH: blk * N + (qn + 1) * QCH],
                    cell[0][:m, 0:QCH])
            return [mm0, mm1, drain]

        def v_group(nb):
            ps = psM.tile([P, QCH], f32, name=f"vps_{nb}", tag="psM")
            for cb in range(CB):
                nc.tensor.matmul(
                    ps[:, 0:192],
                    lhsT=xT_sb[cb][:, nb * P:(nb + 1) * P],
                    rhs=wvT_sb[:, cb * 192:(cb + 1) * 192],
                    start=(cb == 0), stop=(cb == CB - 1),
                )
            vv = v_sb[:, nb * VW:(nb + 1) * VW].rearrange("p (h w) -> p h w", h=HPC)
            pp = ps[:, 0:192].rearrange("p (h w) -> p h w", h=HPC)
            nc.vector.tensor_copy(vv[:, :, 0:64], pp[:, :, :])
            nc.vector.memset(vv[:, :, 64:65], 1.0)

        # ---- lead-in: k0/k1 (all kb), q0/q1 chunks 0-1, all of V --------
        co, m, dst, blk = PASS_K01
        for qp in range(2):
            qk_group_psS(co, m, dst, blk, qp)
        co, m, dst, blk = PASS_Q01
        qk_group_psS(co, m, dst, blk, 0)
        for nb in range(KB):
            v_group(nb)

        # ---- deferred extras (consumed one per attention iteration);
        # ordered by deadline: q0/q1 chunks 2-3 by iter 16, q2/k2 by 64.
        extras = []
        co, m, dst, blk = PASS_Q01
        for qn in (2, 3):
            extras.extend(qk_group512(co, m, dst, blk, qn))
        for co, m, dst, blk in (PASS_Q2, PASS_K2):
            for qn in range(4):
                extras.extend(qk_group512(co, m, dst, blk, qn))

        front = []

        def consume():
            if front:
                front.pop(0)()
            elif extras:
                extras.pop(0)()

        # ---- attention helpers -----------------------------------------
        def make_pv(h, cps, kb, es):
            def run():
                for qx in range(2):
                    nc.tensor.matmul(
                        cps[qx][0:65, :],
                        lhsT=v_sb[:, kb * VW + h * 65: kb * VW + (h + 1) * 65],
                        rhs=es[:, qx * QCH:(qx + 1) * QCH],
                        start=(kb == 0), stop=(kb == KB - 1),
                    )
            return run

        def norm_closures(h, f, cps):
            """Normalize the finished half into ctxT_sb.  bps tiles are
            allocated eagerly so the psC ring order (cps0, cps1, bps0, bps1
            per half) matches consumption order."""
            rec_in = smallp.tile([2, QCH], f32, name=f"rin_{h}_{f}", tag="rin")
            lt = smallp.tile([2, QCH], f32, name=f"lt_{h}_{f}", tag="lt")
            rec = smallp.tile([2, QCH], bf16, name=f"rec_{h}_{f}", tag="rec")
            rb = [smallp.tile([64, QCH], bf16, name=f"rb{qx}_{h}_{f}", tag=f"rb{qx}")
                  for qx in range(2)]
            bps0 = psC.tile([65, QCH], f32, name=f"bps0_{h}_{f}", tag="psC")
            bps1 = psC.tile([65, QCH], f32, name=f"bps1_{h}_{f}", tag="psC")

            def denoms():
                for qx in range(2):
                    nc.vector.tensor_copy(
                        dtmp_sb[64:65, qx * QCH:(qx + 1) * QCH],
                        cps[qx][64:65, :])
                nc.sync.dma_start(rec_in[0:1, :], dtmp_sb[64:65, 0:QCH])
                nc.sync.dma_start(rec_in[1:2, :], dtmp_sb[64:65, QCH:2 * QCH])

            def lnexp():
                nc.scalar.activation(lt, rec_in, Log)
                nc.scalar.activation(rec, lt, Exp, scale=-1.0)
                nc.tensor.matmul(bps0[0:64, :], lhsT=sel_sb[:, 0:64], rhs=rec[:, :],
                                 start=True, stop=True)

            def mul0():
                qc = 2 * f
                nc.vector.tensor_copy(rb[0][:, :], bps0[0:64, :])
                nc.tensor.matmul(bps1[0:64, :], lhsT=sel_sb[:, 64:128], rhs=rec[:, :],
                                 start=True, stop=True)
                nc.vector.tensor_mul(
                    ctxT_sb[0:64, h * N + qc * QCH: h * N + (qc + 1) * QCH],
                    cps[0][0:64, :], rb[0][:, :])

            def mul1():
                qc = 2 * f + 1
                nc.vector.tensor_copy(rb[1][:, :], bps1[0:64, :])
                nc.vector.tensor_mul(
                    ctxT_sb[0:64, h * N + qc * QCH: h * N + (qc + 1) * QCH],
                    cps[1][0:64, :], rb[1][:, :])
                # the previous half's psC slots are now fully released:
                # PV pops for the new half may resume (see boundary_ok)
                boundary_ok[0] = True
            return [denoms, lnexp, mul0, mul1]

        def a2a_stage(h):
            # ctxf row = s4*32 + dl, block dh at cols dh*512 (dh-major so
            # the last head's A2A can be split into two d-row pieces).
            # Recvs ride the gpsimd queue -- on sync they head-of-line
            # block the next boundary's DMAs behind the collective sem.
            send_h = dramp.tile([NCORES, 64, RQ], bf16, name=f"send_{h}")
            recv_h = dramp.tile([NCORES, 64, RQ], bf16, name=f"recv_{h}")
            ctxf = ctxfp.tile([P, 2 * QCH], bf16, name=f"ctxf_{h}", tag="ctxf")

            def stage():
                for j in range(NCORES):
                    nc.sync.dma_start(send_h[j, :, :],
                                      ctxT_sb[:, h * N + j * RQ: h * N + (j + 1) * RQ])
                nc.gpsimd.collective_compute(
                    "AllToAll", mybir.AluOpType.bypass,
                    replica_groups=[list(range(NCORES))],
                    ins=[send_h.opt()], outs=[recv_h.opt()],
                )
                for s in range(NCORES):
                    s4 = s % 4
                    for dh in range(2):
                        nc.gpsimd.dma_start(
                            ctxf[s4 * 32: s4 * 32 + 32,
                                 dh * QCH + (s // 4) * RQ: dh * QCH + (s // 4) * RQ + RQ],
                            recv_h[s, dh * 32:(dh + 1) * 32, :])
            return stage, ctxf

        def outproj_closures(r, ctxf, src_acc, dst_acc):
            cls = []
            for cbo in range(CB):
                def run(cbo=cbo):
                    ps = psM.tile([P, QCH], f32, name=f"ops_{r}_{cbo}", tag="psM")
                    for jb in range(2):
                        nc.tensor.matmul(
                            ps[:, :],
                            lhsT=woTp_sb[:, (2 * r + jb) * C + cbo * P: (2 * r + jb) * C + (cbo + 1) * P],
                            rhs=ctxf[:, jb * QCH:(jb + 1) * QCH],
                            start=(jb == 0), stop=(jb == 1),
                        )
                    nc.vector.tensor_add(
                        dst_acc[:, cbo * QCH:(cbo + 1) * QCH],
                        ps[:, :], src_acc[:, cbo * QCH:(cbo + 1) * QCH])
                cls.append(run)
            return cls

        # ---- main attention loop ---------------------------------------
        acc_chain = [(bias_bc, accB), (accB, accA)]
        pending_a2a = []  # (round, ctxf, release_iter)
        iter_no = 0
        last_cps = None
        # PV matmuls for a new half may only be emitted once the previous
        # half's normalization (which releases the psC ring slots via PE
        # broadcast matmuls emitted in the norm closures) is in the stream;
        # otherwise the in-order PE queue deadlocks on the slot wait.
        boundary_ok = [True]

        for h in range(HPC):
            hb_, ho_ = HOFF[h]
            for f in range(2):
                cps = [psC.tile([65, QCH], f32, name=f"cps_{h}_{f}_{qx}", tag="psC")
                       for qx in range(2)]
                pvq = []
                for kb in range(KB):
                    sps = psS.tile([P, 2 * QCH], f32, name=f"sps_{h}_{f}_{kb}", tag="psS")
                    for qx in range(2):
                        qc = 2 * f + qx
                        nc.tensor.matmul(
                            sps[:, qx * QCH:(qx + 1) * QCH],
                            lhsT=kT_sb[ho_:ho_ + 64, hb_ * N + kb * P: hb_ * N + (kb + 1) * P],
                            rhs=qT_sb[ho_:ho_ + 64, hb_ * N + qc * QCH: hb_ * N + qc * QCH + QCH],
                            start=True, stop=True,
                        )
                    es = esp.tile([P, 2 * QCH], bf16, name=f"es_{h}_{f}_{kb}", tag="es")
                    nc.scalar.activation(es, sps, Exp, scale=SCALE)
                    pvq.append(make_pv(h, cps, kb, es))
                    if pending_a2a and iter_no >= pending_a2a[0][2]:
                        r, ctxf, _ = pending_a2a.pop(0)
                        src, dst = acc_chain[r]
                        front.extend(outproj_closures(r, ctxf, src, dst))
                    consume()
                    # drain boundary closures at double rate right after a
                    # half boundary: the denominator->ln->exp chain feeds
                    # the ACT stream, and every iteration it waits is an
                    # ACT stall
                    if kb < 4 and front:
                        consume()
                    lag = 2 if (h == HPC - 1 and f == 1) else PVLAG
                    if len(pvq) > lag and boundary_ok[0]:
                        pvq.pop(0)()
                    iter_no += 1
                # end of half: flush the pending PVs two per slot, then
                # normalization (+ A2A staging when the head is done).
                for i in range(0, len(pvq), 2):
                    pair = pvq[i:i + 2]
                    front.append(lambda pair=pair: [c() for c in pair])
                pvq = []
                if h == HPC - 1 and f == 1:
                    last_cps = cps  # tail handles the final norm + A2A
                else:
                    boundary_ok[0] = False
                    front.extend(norm_closures(h, f, cps))
                    if f == 1:
                        stage, ctxf = a2a_stage(h)
                        front.append(stage)
                        pending_a2a.append((h, ctxf, iter_no + OUTPROJ_DELAY[h]))

        # ---- tail: final norm split by d-half, A2A in two pipelined
        # pieces, sends/recvs/stores spread over the idle queues ----------
        while front or extras:
            consume()
        r = HPC - 1
        cps = last_cps
        rec_in = smallp.tile([2, QCH], f32, name="rin_T", tag="rin")
        lt = smallp.tile([2, QCH], f32, name="lt_T", tag="lt")
        rec = smallp.tile([2, QCH], bf16, name="rec_T", tag="rec")
        rb = [smallp.tile([64, QCH], bf16, name=f"rb{qx}_T", tag=f"rb{qx}")
              for qx in range(2)]
        bps0 = psC.tile([65, QCH], f32, name="bps0_T", tag="psC")
        bps1 = psC.tile([65, QCH], f32, name="bps1_T", tag="psC")
        for qx in range(2):
            nc.vector.tensor_copy(dtmp_sb[64:65, qx * QCH:(qx + 1) * QCH],
                                  cps[qx][64:65, :])
        nc.sync.dma_start(rec_in[0:1, :], dtmp_sb[64:65, 0:QCH])
        nc.sync.dma_start(rec_in[1:2, :], dtmp_sb[64:65, QCH:2 * QCH])
        nc.scalar.activation(lt, rec_in, Log)
        nc.scalar.activation(rec, lt, Exp, scale=-1.0)
        nc.tensor.matmul(bps0[0:64, :], lhsT=sel_sb[:, 0:64], rhs=rec[:, :],
                         start=True, stop=True)
        nc.vector.tensor_copy(rb[0][:, :], bps0[0:64, :])
        nc.tensor.matmul(bps1[0:64, :], lhsT=sel_sb[:, 64:128], rhs=rec[:, :],
                         start=True, stop=True)
        nc.vector.tensor_copy(rb[1][:, :], bps1[0:64, :])

        ctxf_t = ctxfp.tile([P, 2 * QCH], bf16, name="ctxf_T", tag="ctxf")
        send_p = [dramp.tile([NCORES, 32, RQ], bf16, name=f"sendT_{dh}") for dh in range(2)]
        recv_p = [dramp.tile([NCORES, 32, RQ], bf16, name=f"recvT_{dh}") for dh in range(2)]
        for dh in range(2):
            # normalize only this d-half's 32 rows, stage, and trigger --
            # the lo piece's collective starts while the hi rows multiply
            for qx in range(2):
                qc = 2 + qx
                nc.vector.tensor_mul(
                    ctxT_sb[dh * 32:(dh + 1) * 32, r * N + qc * QCH: r * N + (qc + 1) * QCH],
                    cps[qx][dh * 32:(dh + 1) * 32, :], rb[qx][dh * 32:(dh + 1) * 32, :])
            for j in range(NCORES):
                eng = nc.sync if j % 2 == 0 else nc.scalar
                eng.dma_start(send_p[dh][j, :, :],
                              ctxT_sb[dh * 32:(dh + 1) * 32, r * N + j * RQ: r * N + (j + 1) * RQ])
            nc.gpsimd.collective_compute(
                "AllToAll", mybir.AluOpType.bypass,
                replica_groups=[list(range(NCORES))],
                ins=[send_p[dh].opt()], outs=[recv_p[dh].opt()],
            )
        for dh in range(2):
            engs = [nc.gpsimd, nc.scalar, nc.sync]
            for s in range(NCORES):
                s4 = s % 4
                engs[s % 3].dma_start(
                    ctxf_t[s4 * 32: s4 * 32 + 32,
                           dh * QCH + (s // 4) * RQ: dh * QCH + (s // 4) * RQ + RQ],
                    recv_p[dh][s, :, :])
            src = accA if dh == 0 else accB
            for cbo in range(CB):
                ps = psS.tile([P, 2 * QCH], f32, name=f"opsT_{dh}_{cbo}", tag="psS")
                nc.tensor.matmul(
                    ps[:, 0:QCH],
                    lhsT=woTp_sb[:, (2 * r + dh) * C + cbo * P: (2 * r + dh) * C + (cbo + 1) * P],
                    rhs=ctxf_t[:, dh * QCH:(dh + 1) * QCH],
                    start=True, stop=True,
                )
                if dh == 0:
                    nc.vector.tensor_add(
                        accB[:, cbo * QCH:(cbo + 1) * QCH],
                        ps[:, 0:QCH], src[:, cbo * QCH:(cbo + 1) * QCH])
                else:
                    osb = outp.tile([P, QCH], f32, name=f"osb_{cbo}", tag="osb")
                    nc.vector.tensor_add(osb, ps[:, 0:QCH], src[:, cbo * QCH:(cbo + 1) * QCH])
                    eng = nc.sync if cbo % 2 == 0 else nc.scalar
                    eng.dma_start(out_d[cbo * P:(cbo + 1) * P, :], osb)


_JOINT_SET = "natural_log_exp_and_others"


def _pin_act_tables():
    """Restrict the activation-table chooser to the one set containing
    exp+ln+identity, so the compiled stream has a single ACT_TABLE_LOAD
    instead of flip-flopping between exp_and_others and natural_log (a
    ~1.3us stall at every half boundary).  Only the table *selection* for
    this kernel's own compile is affected; list positions (the
    act_func_set_id encoding) are preserved."""
    orig = bacc.get_activation_tables

    def pinned(arch):
        t = orig(arch)
        if _JOINT_SET not in t:
            return t
        return {k: (v if k == _JOINT_SET else set()) for k, v in t.items()}

    bacc.get_activation_tables = pinned
    return orig


def build():
    orig_tables = _pin_act_tables()
    try:
        return _build()
    finally:
        bacc.get_activation_tables = orig_tables


def _build():
    nc = bacc.Bacc("TRN2", target_bir_lowering=False, debug=False, num_devices=NCORES)
    xT = nc.dram_tensor("xT", [C, N], bf16, kind="ExternalInput").ap()
    wqkT = nc.dram_tensor("wqkT", [C, 2 * HPC * HD], bf16, kind="ExternalInput").ap()
    wvT = nc.dram_tensor("wvT", [C, HPC * HD], bf16, kind="ExternalInput").ap()
    woTp = nc.dram_tensor("woTp", [C, C], bf16, kind="ExternalInput").ap()
    bo_d = nc.dram_tensor("bo", [C, 1], f32, kind="ExternalInput").ap()
    out_d = nc.dram_tensor("out", [C, 2 * RQ], f32, kind="ExternalOutput").ap()
    with tile.TileContext(nc) as tc:
        _body(nc, tc, xT, wqkT, wvT, woTp, bo_d, out_d)
    nc.compile()
    return nc


_NC = None


def _get_nc():
    global _NC
    if _NC is None:
        _NC = build()
    return _NC


# Wo row permutation (dh-major): A2A round r, d-half dh delivers global
# heads {s4*3+r} rows [dh*32,(dh+1)*32) as contiguous channel block
# r*256 + dh*128 + s4*32 + dl.
_PERM = np.array([(s4 * 3 + r) * 64 + dh * 32 + dl
                  for r in range(HPC) for dh in range(2)
                  for s4 in range(4) for dl in range(32)])


def make_in_maps(x, Wq, Wk, Wv, Wo, bo):
    x = np.asarray(x, np.float32)
    woTp = np.ascontiguousarray(np.asarray(Wo, np.float32).T[_PERM, :]).astype(ml_dtypes.bfloat16)
    bo_col = np.ascontiguousarray(np.asarray(bo, np.float32).reshape(C, 1))
    in_maps = []
    for i in range(NCORES):
        b = i // 4
        hs = (i % 4) * HPC
        rq = slice(hs * HD, (hs + HPC) * HD)
        wq_s = np.asarray(Wq, np.float32)[rq]  # [192, 768]
        wk_s = np.asarray(Wk, np.float32)[rq]
        # column order: [q0 q1 | k0 k1 | q2 | k2]
        wqk = np.concatenate([wq_s[0:128], wk_s[0:128], wq_s[128:192], wk_s[128:192]], axis=0).T
        in_maps.append({
            "xT": np.ascontiguousarray(x[b].T).astype(ml_dtypes.bfloat16),
            "wqkT": np.ascontiguousarray(wqk).astype(ml_dtypes.bfloat16),
            "wvT": np.ascontiguousarray(np.asarray(Wv, np.float32)[rq].T).astype(ml_dtypes.bfloat16),
            "woTp": woTp,
            "bo": bo_col,
        })
    return in_maps


def unshard(results):
    out = np.empty((B, N, C), np.float32)
    for i, r in enumerate(results):
        o = r["out"]  # [768, 512]: cols 0-255 batch 0, 256-511 batch 1
        out[0, i * RQ:(i + 1) * RQ, :] = o[:, :RQ].T
        out[1, i * RQ:(i + 1) * RQ, :] = o[:, RQ:].T
    return out


def kernel(x, Wq, Wk, Wv, Wo, bo):
    nc = _get_nc()
    in_maps = make_in_maps(x, Wq, Wk, Wv, Wo, bo)
    res = run_bass_kernel_spmd(nc, in_maps, core_ids=list(range(NCORES)))
    return unshard(res.results)
